# revision 2
# baseline (speedup 1.0000x reference)
"""Trainium2 Bass kernel for nn_Attention_kv (dense transformer block).

Sharding: data-parallel over batch B=8 across the 8 NeuronCores — one batch
element per core, no collectives (host scatters inputs / stacks outputs).

Valid-row compaction (host side): ~50% of sequence positions have mask==0.
Masked QUERY rows of both attentions get uniform attention over ALL keys, so
every masked row of the final output equals one per-batch vector
  fill[b] = (mean_M(text_x[b]) @ Wkv[:,C:] + bkv[C:]) @ Wffn + bffn
computable on the host in microseconds.  Valid rows never depend on masked
rows: an invalid key's attention weight is exp(s*scale - 10000) == 0.0 in
fp32 (hard underflow), identical to the reference's jnp.where(-10000) +
softmax.  So the device only sees the gathered valid rows, padded to a
multiple of 128 (634 max over batches -> MV=640 for the graded inputs), with
a recomputed 1/0 mask for the pad tail; the host scatters device rows back
into the valid positions and broadcast-fills masked rows with fill[b].
This removes ~48% of all PE work (projections scale by MV/M, attention by
(MV/M)^2) and is exact up to fp32 reassociation.

Per-core pipeline (seq MV, dim C=768), unchanged from the full-size design:
  x^T, t^T via PE 128x128 transposes
  -> qkv projection (q^T, k^T produced transposed [d, seq]; v natural)
  -> attn1: scores computed TRANSPOSED S^T[sk, sq]; max-free masked softmax
     (additive -10000 key mask + multiplicative query-mask zeroing); row
     sums over partitions via PE ones-matmul; out^T accumulated across 6
     PSUM banks flash-style; normalization DEFERRED into the next phase's
     PSUM copyback
  -> cq projection -> kv projection (from text) -> attn2 -> ffn -> out.

All matmuls run in float32r (TF32-like PE datapath, 1 cycle/row — measured
numerically identical to the fp32 4-cycle/row path on this hardware).
"""

import sys

sys.path.insert(0, "/opt/trn_rl_repo")

from contextlib import ExitStack

import numpy as np

import concourse.bass as bass
import concourse.mybir as mybir
import concourse.tile as tile
from concourse import bacc
from concourse.bass_utils import run_bass_kernel_spmd
from concourse.masks import make_identity

P = 128
M = 1024  # full sequence length per batch element (host side)
C = 768  # model dim
KT = C // P  # 6 contraction tiles
MV_DEFAULT = 640  # compacted device seq len for the graded inputs
SCALE = float(C) ** -0.5
NEG = -10000.0

F32 = mybir.dt.float32
F32R = mybir.dt.float32r
AL = mybir.AluOpType
AF = mybir.ActivationFunctionType

N_CORES = 8


def _qchunks(mv):
    """Split the device seq dim into free-dim chunks of <= 512 (one PSUM
    bank of fp32)."""
    out = []
    off = 0
    while off < mv:
        w = min(512, mv - off)
        out.append((off, w))
        off += w
    return out


def _proj_natural(nc, lhs_src, w_rhs, dst, bias_bc, psum_pool, mt):
    """dst[:, i, :] (shape [P, mt, C]) = src @ W + bias.

    lhs_src: AP [P, KT, MV] (x^T layout, f32r) -- lhsT tiles [P, 128]
    w_rhs: AP [P, KT, C] (weight, f32r) -- rhs tiles [P, chunk]
    bias_bc: AP [P, C] broadcast bias or None
    """
    chunks = [(0, 512), (512, 256)]
    for i in range(mt):
        pss = []
        for (off, w) in chunks:
            ps = psum_pool.tile([P, 512], F32, tag="st", name=f"ps_v_{i}_{off}")
            for a in range(KT):
                nc.tensor.matmul(
                    ps[:, :w],
                    lhs_src[:, a, i * P : (i + 1) * P],
                    w_rhs[:, a, off : off + w],
                    start=(a == 0),
                    stop=(a == KT - 1),
                )
            pss.append(ps)
        for (off, w), ps in zip(chunks, pss):
            if bias_bc is not None:
                nc.any.tensor_add(
                    out=dst[:, i, off : off + w],
                    in0=ps[:, :w],
                    in1=bias_bc[:, off : off + w],
                )
            else:
                nc.any.tensor_copy(out=dst[:, i, off : off + w], in_=ps[:, :w])


def _attention(nc, io, psum_pool, qT, kT, vn, outT, colb, rm_scaled,
               ones_r, ones_row_r, label, mt, qch,
               recip_col=None, dram_pool=None):
    """outT[:, d, :] = (UNNORMALIZED attn numerator)^T, [P, KT, MV] f32r.

    Normalization is deferred to the consumer: returns per-chunk rbc
    broadcast tile slices [P, fw] (1/rowsum along free sq) unless recip_col
    is given, in which case recip values are instead written into
    recip_col ([P, mt] column layout) and no bcast is made.

    qT, kT: [P, KT, MV] f32r (d on partitions); vn: [P, mt, C] f32r.
    colb: [P, mt] fp32 = (mask-1)*10000/scale along sk partitions.
    rm_scaled: [P, MV] fp32 = mask*scale broadcast (varies along free sq).
    """
    rbcs = []
    for c, (off, fw) in enumerate(qch):
        sq = slice(off, off + fw)
        # out^T accumulators: 6 banks
        pos = [
            psum_pool.tile([P, 512], F32, tag="po", name=f"po_{label}_{c}_{d}")
            for d in range(KT)
        ]
        p_tiles = []
        prev = None  # (j, p_j) pending out^T matmuls
        for j in range(mt):
            st = psum_pool.tile([P, 512], F32, tag="st", name=f"st_{label}_{c}_{j}")
            for a in range(KT):
                nc.tensor.matmul(
                    st[:, :fw],
                    kT[:, a, j * P : (j + 1) * P],
                    qT[:, a, sq],
                    start=(a == 0),
                    stop=(a == KT - 1),
                )
            # masked = (S^T + colb_j) * rm_scaled ; exp
            mk = io.tile([P, 512], F32, tag="mk", name=f"mk_{label}_{c}_{j}", bufs=2)
            nc.vector.scalar_tensor_tensor(
                out=mk[:, :fw],
                in0=st[:, :fw],
                scalar=colb[:, j : j + 1],
                in1=rm_scaled[:, sq],
                op0=AL.add,
                op1=AL.mult,
            )
            pj = io.tile([P, 512], F32R, tag="pp", name=f"p_{label}_{c}_{j}", bufs=mt + 2)
            nc.scalar.activation(pj[:, :fw], mk[:, :fw], AF.Exp)
            p_tiles.append(pj)
            if prev is not None:
                jj, pprev = prev
                for d in range(KT):
                    nc.tensor.matmul(
                        pos[d][:, :fw],
                        vn[:, jj, d * P : (d + 1) * P],
                        pprev[:, :fw],
                        start=(jj == 0),
                        stop=False,
                    )
            prev = (j, pj)
        jj, pprev = prev
        for d in range(KT):
            nc.tensor.matmul(
                pos[d][:, :fw],
                vn[:, jj, d * P : (d + 1) * P],
                pprev[:, :fw],
                start=(jj == 0),
                stop=True,
            )
        # row sums over sk (partitions + tiles) via ones-matmul
        rs = psum_pool.tile([P, 512], F32, tag="st", name=f"rs_{label}_{c}")
        for j in range(mt):
            nc.tensor.matmul(
                rs[0:1, :fw],
                ones_r[:],
                p_tiles[j][:, :fw],
                start=(j == 0),
                stop=(j == mt - 1),
            )
        recip = io.tile([1, 512], F32R, tag="recip", name=f"recip_{label}_{c}", bufs=2)
        with nc.allow_low_precision(reason="f32r recip feeds f32r bcast matmul"):
            nc.vector.reciprocal(recip[:, :fw], rs[0:1, :fw])
        if recip_col is None:
            # broadcast recip across partitions via K=1 f32r matmul
            bc = psum_pool.tile([P, 512], F32, tag="st", name=f"bc_{label}_{c}")
            nc.tensor.matmul(bc[:, :fw], ones_row_r[:], recip[:, :fw], start=True, stop=True)
            rbc = io.tile([P, 512], F32, tag="rbc", name=f"rbc_{label}_{c}", bufs=4)
            nc.vector.tensor_copy(out=rbc[:, :fw], in_=bc[:, :fw])
            rbcs.append(rbc[:, :fw])
        else:
            # column layout recip_col[p, a] = 1/rowsum[sq = off + a*P + p]
            # via a DRAM bounce (free->partition reshuffles need DMA via DRAM)
            scr = dram_pool.tile([1, 512], F32, tag="rscr", name=f"rscr_{label}_{c}", bufs=2)
            nc.sync.dma_start(scr[:, :fw], recip[:, :fw].bitcast(F32))
            nc.sync.dma_start(
                recip_col[:, off // P : (off + fw) // P],
                scr[0, :fw].rearrange("(a p) -> p a", p=P),
            )
        # UNNORMALIZED copyback (releases psum_o banks immediately)
        for d in range(KT):
            nc.vector.tensor_copy(out=outT[:, d, sq], in_=pos[d][:, :fw])
    return rbcs


def _transpose_in(nc, io, psum_tr, src_dram, dst, ident, tag, mt):
    """dst [P, KT, MV] (f32r) = src^T, via PE 128x128 transposes."""
    for i in range(mt):
        xin = io.tile([P, C], F32R, tag="xin", name=f"xin_{tag}_{i}", bufs=3)
        nc.sync.dma_start(xin[:], src_dram[i * P : (i + 1) * P, :])
        for a in range(KT):
            tr = psum_tr.tile([P, P], F32R, tag="tr", name=f"tr_{tag}_{i}_{a}")
            nc.tensor.transpose(tr[:], xin[:, a * P : (a + 1) * P], ident[:])
            nc.any.tensor_copy(out=dst[:, a, i * P : (i + 1) * P], in_=tr[:])


def build_nc(n_iters=1, mv=MV_DEFAULT):
    mt = mv // P
    qch = _qchunks(mv)

    nc = bacc.Bacc(trn_type="TRN2", target_bir_lowering=False, debug=False)

    x_d = nc.dram_tensor("x", [mv, C], F32R, kind="ExternalInput").ap()
    t_d = nc.dram_tensor("t", [mv, C], F32R, kind="ExternalInput").ap()
    mask_d = nc.dram_tensor("mask", [1, mv], F32, kind="ExternalInput").ap()
    wqkv_d = nc.dram_tensor("Wqkv", [C, 3 * C], F32R, kind="ExternalInput").ap()
    bqkv_d = nc.dram_tensor("bqkv", [1, 3 * C], F32, kind="ExternalInput").ap()
    wq_d = nc.dram_tensor("Wq", [C, C], F32R, kind="ExternalInput").ap()
    bq_d = nc.dram_tensor("bq", [1, C], F32, kind="ExternalInput").ap()
    wkv_d = nc.dram_tensor("Wkv", [C, 2 * C], F32R, kind="ExternalInput").ap()
    bkv_d = nc.dram_tensor("bkv", [1, 2 * C], F32, kind="ExternalInput").ap()
    wffn_d = nc.dram_tensor("Wffn", [C, C], F32R, kind="ExternalInput").ap()
    bffn_d = nc.dram_tensor("bffn", [1, C], F32, kind="ExternalInput").ap()
    out_d = nc.dram_tensor("out", [mv, C], F32, kind="ExternalOutput").ap()

    wqkv_t = wqkv_d.rearrange("(a p) n -> p a n", p=P)  # [P, KT, 3C]
    wq_t = wq_d.rearrange("(a p) n -> p a n", p=P)
    wkv_t = wkv_d.rearrange("(a p) n -> p a n", p=P)
    wffn_t = wffn_d.rearrange("(a p) n -> p a n", p=P)

    with tile.TileContext(nc) as tc, ExitStack() as ctx:
        const = ctx.enter_context(tc.tile_pool(name="const", bufs=1))
        acts = ctx.enter_context(tc.tile_pool(name="acts", bufs=1))
        wpool = ctx.enter_context(tc.tile_pool(name="wpool", bufs=1))
        io = ctx.enter_context(tc.tile_pool(name="io", bufs=1))
        psum_main = ctx.enter_context(tc.tile_pool(name="psum_main", bufs=2, space="PSUM"))

        # ---- constants ----
        ident32 = const.tile([P, P], F32, tag="ident32", name="ident32")
        make_identity(nc, ident32[:])
        ident = const.tile([P, P], F32R, tag="ident", name="ident")
        nc.vector.tensor_copy(out=ident[:], in_=ident32[:])

        mask_t = const.tile([P, mt], F32, tag="mask_t", name="mask_t")
        nc.sync.dma_start(mask_t[:], mask_d[0].rearrange("(a p) -> p a", p=P))
        colb = const.tile([P, mt], F32, tag="colb", name="colb")
        nc.vector.tensor_scalar(
            colb[:], mask_t[:], 10000.0 / SCALE, -10000.0 / SCALE, AL.mult, AL.add
        )

        rm_scaled = const.tile([P, mv], F32, tag="rm_scaled", name="rm_scaled")
        nc.sync.dma_start(rm_scaled[:], mask_d.partition_broadcast(P))
        nc.vector.tensor_scalar_mul(rm_scaled[:], rm_scaled[:], SCALE)

        ones32 = const.tile([P, 1], F32, tag="ones32", name="ones32")
        nc.gpsimd.memset(ones32[:], 1.0)
        ones_r = const.tile([P, 1], F32R, tag="ones_r", name="ones_r")
        nc.vector.tensor_copy(out=ones_r[:], in_=ones32[:])
        ones_row32 = const.tile([1, P], F32, tag="ones_row32", name="ones_row32")
        nc.gpsimd.memset(ones_row32[:], 1.0)
        ones_row_r = const.tile([1, P], F32R, tag="ones_row_r", name="ones_row_r")
        nc.vector.tensor_copy(out=ones_row_r[:], in_=ones_row32[:])

        # per-partition bias columns (d on partitions)
        bq_col = const.tile([P, KT], F32, tag="bq_col", name="bq_col")
        nc.sync.dma_start(bq_col[:], bqkv_d[0, 0:C].rearrange("(a p) -> p a", p=P))
        bk_col = const.tile([P, KT], F32, tag="bk_col", name="bk_col")
        nc.sync.dma_start(bk_col[:], bqkv_d[0, C : 2 * C].rearrange("(a p) -> p a", p=P))
        bcq_col = const.tile([P, KT], F32, tag="bcq_col", name="bcq_col")
        nc.sync.dma_start(bcq_col[:], bq_d[0, :].rearrange("(a p) -> p a", p=P))
        bck_col = const.tile([P, KT], F32, tag="bck_col", name="bck_col")
        nc.sync.dma_start(bck_col[:], bkv_d[0, 0:C].rearrange("(a p) -> p a", p=P))

        # ---- big activation tensors ----
        for _it in range(n_iters):
            _body_iter(nc, tc, ctx, acts, wpool, io, const, psum_main,
                       x_d, t_d, wqkv_t, wq_t, wkv_t, wffn_t,
                       bqkv_d, bq_d, bkv_d, bffn_d, out_d,
                       ident, colb, rm_scaled, ones_r, ones_row_r,
                       bq_col, bk_col, bcq_col, bck_col, _it, mt, qch, mv)

    nc.compile()
    return nc


def _body_iter(nc, tc, ctx, acts, wpool, io, const, psum_main,
               x_d, t_d, wqkv_t, wq_t, wkv_t, wffn_t,
               bqkv_d, bq_d, bkv_d, bffn_d, out_d,
               ident, colb, rm_scaled, ones_r, ones_row_r,
               bq_col, bk_col, bcq_col, bck_col, it, mt, qch, mv):
    if True:
        xT = acts.tile([P, KT, mv], F32R, tag="xT", name="xT")  # x^T
        qT = acts.tile([P, KT, mv], F32R, tag="qT", name="qT")
        kTt = acts.tile([P, KT, mv], F32R, tag="kT", name="kT")
        vn = acts.tile([P, mt, C], F32R, tag="vn", name="vn")
        o1T = acts.tile([P, KT, mv], F32R, tag="oT", name="o1T")

        # ---- phase A: transpose x ----
        psum_tr = tc.alloc_tile_pool(name="psum_tr", bufs=6, space="PSUM")
        _transpose_in(nc, io, psum_tr, x_d, xT, ident, f"x{it}", mt)

        # ---- phase B: qkv projection ----
        bias_bc = wpool.tile([P, C], F32, tag="bbc", name="vbias_bc")
        nc.sync.dma_start(bias_bc[:], bqkv_d[0:1, 2 * C : 3 * C].partition_broadcast(P))

        for part, (dst, bcol) in enumerate([(qT, bq_col), (kTt, bk_col)]):
            for d in range(KT):
                w = wpool.tile([P, KT, P], F32R, tag="ws", name=f"wsq_{part}_{d}", bufs=3)
                nc.sync.dma_start(
                    w[:],
                    wqkv_t[:, :, part * C + d * P : part * C + (d + 1) * P],
                )
                for (off, fw) in qch:
                    ps = psum_main.tile([P, 512], F32, tag="st", name=f"ps_qk_{part}_{d}_{off}")
                    for a in range(KT):
                        nc.tensor.matmul(
                            ps[:, :fw],
                            w[:, a, :],
                            xT[:, a, off : off + fw],
                            start=(a == 0),
                            stop=(a == KT - 1),
                        )
                    nc.any.tensor_scalar_add(
                        dst[:, d, off : off + fw], ps[:, :fw], bcol[:, d : d + 1]
                    )

        vw = wpool.tile([P, KT, C], F32R, tag="vw", name="vw_qkv")
        nc.sync.dma_start(vw[:], wqkv_t[:, :, 2 * C : 3 * C])
        _proj_natural(nc, xT, vw, vn, bias_bc, psum_main, mt)

        # ---- phase A2: transpose t (reuses xT slot) ----
        tT = acts.tile([P, KT, mv], F32R, tag="xT", name="tT")
        _transpose_in(nc, io, psum_tr, t_d, tT, ident, f"t{it}", mt)
        psum_tr.release()

        psum_att = tc.alloc_tile_pool(name="psum_att", bufs=6, space="PSUM")

        # ---- phase C/D: attention 1 ----
        class _AttPsum:
            def tile(self, shape, dtype, tag, name):
                pool = psum_att if tag == "po" else psum_main
                return pool.tile(shape, dtype, tag=tag, name=name)

        att_psum = _AttPsum()
        rbcs1 = _attention(
            nc, io, att_psum, qT, kTt, vn, o1T, colb, rm_scaled,
            ones_r, ones_row_r, "a1", mt, qch,
        )

        # ---- phase E: cq projection (into qT slot) ----
        cqT = acts.tile([P, KT, mv], F32R, tag="qT", name="cqT")
        wqs = wpool.tile([P, KT, C], F32R, tag="vw", name="wq_sb")
        nc.sync.dma_start(wqs[:], wq_t[:])
        for d in range(KT):
            for ci, (off, fw) in enumerate(qch):
                ps = psum_main.tile([P, 512], F32, tag="st", name=f"ps_cq_{d}_{off}")
                for a in range(KT):
                    nc.tensor.matmul(
                        ps[:, :fw],
                        wqs[:, a, d * P : (d + 1) * P],
                        o1T[:, a, off : off + fw],
                        start=(a == 0),
                        stop=(a == KT - 1),
                    )
                dst = cqT[:, d, off : off + fw]
                nc.any.tensor_mul(out=dst, in0=ps[:, :fw], in1=rbcs1[ci])
                nc.any.tensor_scalar_add(dst, dst, bcq_col[:, d : d + 1])

        # ---- phase F: kv projection from t (into kT, vn slots) ----
        ckT = acts.tile([P, KT, mv], F32R, tag="kT", name="ckT")
        for d in range(KT):
            w = wpool.tile([P, KT, P], F32R, tag="ws", name=f"wsk_{d}", bufs=3)
            nc.sync.dma_start(w[:], wkv_t[:, :, d * P : (d + 1) * P])
            for (off, fw) in qch:
                ps = psum_main.tile([P, 512], F32, tag="st", name=f"ps_ck_{d}_{off}")
                for a in range(KT):
                    nc.tensor.matmul(
                        ps[:, :fw],
                        w[:, a, :],
                        tT[:, a, off : off + fw],
                        start=(a == 0),
                        stop=(a == KT - 1),
                    )
                nc.any.tensor_scalar_add(
                    ckT[:, d, off : off + fw], ps[:, :fw], bck_col[:, d : d + 1]
                )

        cvn = acts.tile([P, mt, C], F32R, tag="vn", name="cvn")
        cvw = wpool.tile([P, KT, C], F32R, tag="vw", name="vw_kv")
        nc.sync.dma_start(cvw[:], wkv_t[:, :, C : 2 * C])
        cv_bias_bc = wpool.tile([P, C], F32, tag="bbc", name="cvbias_bc")
        nc.sync.dma_start(
            cv_bias_bc[:], bkv_d[0:1, C : 2 * C].partition_broadcast(P)
        )
        _proj_natural(nc, tT, cvw, cvn, cv_bias_bc, psum_main, mt)

        # ---- phase G: attention 2 (out2T into xT slot) ----
        o2T = acts.tile([P, KT, mv], F32R, tag="xT", name="o2T")
        recip2_col = io.tile([P, mt], F32, tag="recip2_col", name="recip2_col", bufs=2)
        dram_pool = tc.alloc_tile_pool(name="dram_scr", bufs=1, space="DRAM")
        _attention(
            nc, io, att_psum, cqT, ckT, cvn, o2T, colb, rm_scaled,
            ones_r, ones_row_r, "a2", mt, qch,
            recip_col=recip2_col, dram_pool=dram_pool,
        )
        dram_pool.release()

        # ---- phase H: ffn ----
        wfs = wpool.tile([P, KT, C], F32R, tag="vw", name="wffn_sb")
        nc.sync.dma_start(wfs[:], wffn_t[:])
        ffn_bias_bc = wpool.tile([P, C], F32, tag="bbc", name="ffnbias_bc")
        nc.sync.dma_start(ffn_bias_bc[:], bffn_d[0:1, :].partition_broadcast(P))
        chunks = [(0, 512), (512, 256)]
        for i in range(mt):
            pss = []
            for (off, w) in chunks:
                ps = psum_main.tile([P, 512], F32, tag="st", name=f"ps_f_{i}_{off}")
                for a in range(KT):
                    nc.tensor.matmul(
                        ps[:, :w],
                        o2T[:, a, i * P : (i + 1) * P],
                        wfs[:, a, off : off + w],
                        start=(a == 0),
                        stop=(a == KT - 1),
                    )
                pss.append(ps)
            fin = io.tile([P, C], F32, tag="fin", name=f"fin_{i}", bufs=2)
            for (off, w), ps in zip(chunks, pss):
                nc.vector.scalar_tensor_tensor(
                    out=fin[:, off : off + w],
                    in0=ps[:, :w],
                    scalar=recip2_col[:, i : i + 1],
                    in1=ffn_bias_bc[:, off : off + w],
                    op0=AL.mult,
                    op1=AL.add,
                )
            nc.sync.dma_start(out_d[i * P : (i + 1) * P, :], fin[:])

        psum_att.release()


_NC_CACHE = {}


def _get_nc(mv=MV_DEFAULT):
    if mv not in _NC_CACHE:
        _NC_CACHE[mv] = build_nc(mv=mv)
    return _NC_CACHE[mv]


def prep_inputs(layout_x, text_x, mask, Wqkv, bqkv, Wq, bq, Wkv, bkv, Wffn, bffn):
    """Host-side valid-row compaction.

    Returns (mv, in_maps, idxs, fill) where in_maps feeds the device kernel
    (compacted to mv rows per core), idxs[b] are the valid row indices, and
    fill[b] is the output vector for every masked row of batch b.
    """
    layout_x = np.ascontiguousarray(np.asarray(layout_x, dtype=np.float32))
    text_x = np.ascontiguousarray(np.asarray(text_x, dtype=np.float32))
    mask = np.ascontiguousarray(np.asarray(mask, dtype=np.float32))
    Wqkv = np.ascontiguousarray(np.asarray(Wqkv, dtype=np.float32))
    bqkv = np.ascontiguousarray(np.asarray(bqkv, dtype=np.float32)).reshape(1, 3 * C)
    Wq = np.ascontiguousarray(np.asarray(Wq, dtype=np.float32))
    bq = np.ascontiguousarray(np.asarray(bq, dtype=np.float32)).reshape(1, C)
    Wkv = np.ascontiguousarray(np.asarray(Wkv, dtype=np.float32))
    bkv = np.ascontiguousarray(np.asarray(bkv, dtype=np.float32)).reshape(1, 2 * C)
    Wffn = np.ascontiguousarray(np.asarray(Wffn, dtype=np.float32))
    bffn = np.ascontiguousarray(np.asarray(bffn, dtype=np.float32)).reshape(1, C)

    B = layout_x.shape[0]
    assert B == N_CORES

    idxs = [np.nonzero(mask[b] != 0)[0] for b in range(B)]
    nvs = [len(ix) for ix in idxs]
    mv = min(M, max(P, -(-max(nvs) // P) * P))

    # masked-row output: uniform attn2 over ALL cv rows, then ffn
    tx_mean = text_x.astype(np.float64).mean(axis=1)  # [B, C]
    cv_mean = tx_mean @ Wkv[:, C:].astype(np.float64) + bkv[0, C:].astype(np.float64)
    fill = (cv_mean @ Wffn.astype(np.float64) + bffn[0].astype(np.float64)).astype(
        np.float32
    )  # [B, C]

    in_maps = []
    for b in range(B):
        nv = nvs[b]
        x_c = np.zeros((mv, C), np.float32)
        x_c[:nv] = layout_x[b][idxs[b]]
        t_c = np.zeros((mv, C), np.float32)
        t_c[:nv] = text_x[b][idxs[b]]
        m_c = np.zeros((1, mv), np.float32)
        m_c[0, :nv] = 1.0
        in_maps.append(
            {
                "x": x_c,
                "t": t_c,
                "mask": m_c,
                "Wqkv": Wqkv,
                "bqkv": bqkv,
                "Wq": Wq,
                "bq": bq,
                "Wkv": Wkv,
                "bkv": bkv,
                "Wffn": Wffn,
                "bffn": bffn,
            }
        )
    return mv, in_maps, idxs, fill


def finish_output(dev_outs, idxs, fill):
    """Scatter compacted device outputs back to the full [B, M, C] shape."""
    B = len(idxs)
    out = np.empty((B, M, C), np.float32)
    for b in range(B):
        out[b, :, :] = fill[b]
        out[b, idxs[b], :] = dev_outs[b][: len(idxs[b])]
    return out


def kernel(
    layout_x, text_x, mask, Wqkv, bqkv, Wq, bq, Wkv, bkv, Wffn, bffn
):
    mv, in_maps, idxs, fill = prep_inputs(
        layout_x, text_x, mask, Wqkv, bqkv, Wq, bq, Wkv, bkv, Wffn, bffn
    )
    nc = _get_nc(mv)
    res = run_bass_kernel_spmd(nc, in_maps, core_ids=list(range(N_CORES)))
    return finish_output(
        [res.results[b]["out"] for b in range(N_CORES)], idxs, fill
    )


# revision 23
# speedup vs baseline: 1.8224x; 1.8224x over previous
"""Trainium2 Bass kernel for nn_Attention_kv (dense transformer block).

Sharding: data-parallel over batch B=8 across the 8 NeuronCores — one batch
element per core, no collectives (host scatters inputs / stacks outputs).

Valid-row compaction (host side): ~50% of sequence positions have mask==0.
Masked QUERY rows of both attentions get uniform attention over ALL keys, so
every masked row of the final output equals one per-batch vector
  fill[b] = (mean_M(text_x[b]) @ Wkv[:,C:] + bkv[C:]) @ Wffn + bffn
computable on the host in microseconds.  Valid rows never depend on masked
rows: an invalid key's attention weight is exp(s*scale - 10000) == 0.0 in
fp32 (hard underflow), identical to the reference's jnp.where(-10000) +
softmax.  So the device only sees the gathered valid rows, padded to a
multiple of 128 (634 max over batches -> MV=640 for the graded inputs), with
a recomputed 1/0 mask for the pad tail; the host scatters device rows back
into the valid positions and broadcast-fills masked rows with fill[b].
This removes ~48% of all PE work (projections scale by MV/M, attention by
(MV/M)^2) and is exact up to fp32 reassociation.

Per-core pipeline (seq MV, dim C=768), unchanged from the full-size design:
  x^T, t^T via PE 128x128 transposes
  -> qkv projection (q^T, k^T produced transposed [d, seq]; v natural)
  -> attn1: scores computed TRANSPOSED S^T[sk, sq]; max-free masked softmax
     (additive -10000 key mask + multiplicative query-mask zeroing); row
     sums over partitions via PE ones-matmul; out^T accumulated across 6
     PSUM banks flash-style; normalization DEFERRED into the next phase's
     PSUM copyback
  -> cq projection -> kv projection (from text) -> attn2 -> ffn -> out.

All matmuls run in float32r (TF32-like PE datapath, 1 cycle/row — measured
numerically identical to the fp32 4-cycle/row path on this hardware).
"""

import sys

sys.path.insert(0, "/opt/trn_rl_repo")

from contextlib import ExitStack

import numpy as np

import concourse.bass as bass
import concourse.mybir as mybir
import concourse.tile as tile
from concourse import bacc
from concourse.bass_utils import run_bass_kernel_spmd
from concourse.masks import make_identity

P = 128
M = 1024  # full sequence length per batch element (host side)
C = 768  # model dim
KT = C // P  # 6 contraction tiles
MV_DEFAULT = 640  # compacted device seq len for the graded inputs
MQ_DEFAULT = 544  # compacted query extent (max 534 valid rows per batch)
SCALE = float(C) ** -0.5
NEG = -10000.0

F32 = mybir.dt.float32
F32R = mybir.dt.float32r
AL = mybir.AluOpType
AF = mybir.ActivationFunctionType

N_CORES = 8


def _qchunks(mv):
    """Split the device seq dim into free-dim chunks of <= 512 (one PSUM
    bank of fp32)."""
    out = []
    off = 0
    while off < mv:
        w = min(512, mv - off)
        out.append((off, w))
        off += w
    return out


def _proj_natural(nc, lhs_src, w_rhs, dst, bias_bc, psum_pool, mt):
    """dst[:, i, :] (shape [P, mt, C]) = src @ W + bias.

    lhs_src: AP [P, KT, MV] (x^T layout, f32r) -- lhsT tiles [P, 128]
    w_rhs: AP [P, KT, C] (weight, f32r) -- rhs tiles [P, chunk]
    bias_bc: AP [P, C] broadcast bias or None
    """
    chunks = [(0, 512), (512, 256)]
    for i in range(mt):
        pss = []
        for (off, w) in chunks:
            ps = psum_pool.tile([P, 512], F32, tag="st", name=f"ps_v_{i}_{off}")
            for a in range(KT):
                nc.tensor.matmul(
                    ps[:, :w],
                    lhs_src[:, a, i * P : (i + 1) * P],
                    w_rhs[:, a, off : off + w],
                    start=(a == 0),
                    stop=(a == KT - 1),
                )
            pss.append(ps)
        for (off, w), ps in zip(chunks, pss):
            if bias_bc is not None:
                nc.any.tensor_add(
                    out=dst[:, i, off : off + w],
                    in0=ps[:, :w],
                    in1=bias_bc[:, off : off + w],
                )
            else:
                nc.any.tensor_copy(out=dst[:, i, off : off + w], in_=ps[:, :w])


def _attention(nc, io, psum_pool, qT, kT, vn, outT, colb, rm_scaled,
               ones_r, ones_row_r, label, mt, qch,
               recip_col=None, dram_pool=None):
    # qch are the QUERY chunks and may stop short of the key extent
    # (mt*P): pad queries past max-valid-rows are never consumed.
    """outT[:, d, :] = (UNNORMALIZED attn numerator)^T, [P, KT, MV] f32r.

    Normalization is deferred to the consumer: returns per-chunk rbc
    broadcast tile slices [P, fw] (1/rowsum along free sq) unless recip_col
    is given, in which case recip values are instead written into
    recip_col ([P, mt] column layout) and no bcast is made.

    qT, kT: [P, KT, MV] f32r (d on partitions); vn: [P, mt, C] f32r.
    colb: [P, mt] fp32 = (mask-1)*10000/scale along sk partitions.
    rm_scaled: [P, MV] fp32 = mask*scale broadcast (varies along free sq).
    """
    rbcs = []
    for c, (off, fw) in enumerate(qch):
        sq = slice(off, off + fw)
        # out^T accumulators: 6 banks
        pos = [
            psum_pool.tile([P, 512], F32, tag="po", name=f"po_{label}_{c}_{d}")
            for d in range(KT)
        ]
        p_tiles = []
        prev = None  # (j, p_j) pending out^T matmuls
        for j in range(mt):
            st = psum_pool.tile([P, 512], F32, tag="st", name=f"st_{label}_{c}_{j}")
            for a in range(KT):
                nc.tensor.matmul(
                    st[:, :fw],
                    kT[:, a, j * P : (j + 1) * P],
                    qT[:, a, sq],
                    start=(a == 0),
                    stop=(a == KT - 1),
                )
            # masked = (S^T + colb_j) * rm_scaled ; exp
            mk = io.tile([P, 512], F32, tag="mk", name=f"mk_{label}_{c}_{j}", bufs=2)
            nc.vector.scalar_tensor_tensor(
                out=mk[:, :fw],
                in0=st[:, :fw],
                scalar=colb[:, j : j + 1],
                in1=rm_scaled[:, sq],
                op0=AL.add,
                op1=AL.mult,
            )
            pj = io.tile([P, 512], F32R, tag="pp", name=f"p_{label}_{c}_{j}", bufs=mt + 2)
            nc.scalar.activation(pj[:, :fw], mk[:, :fw], AF.Exp)
            p_tiles.append(pj)
            if prev is not None:
                jj, pprev = prev
                for d in range(KT):
                    nc.tensor.matmul(
                        pos[d][:, :fw],
                        vn[:, jj, d * P : (d + 1) * P],
                        pprev[:, :fw],
                        start=(jj == 0),
                        stop=False,
                    )
            prev = (j, pj)
        jj, pprev = prev
        for d in range(KT):
            nc.tensor.matmul(
                pos[d][:, :fw],
                vn[:, jj, d * P : (d + 1) * P],
                pprev[:, :fw],
                start=(jj == 0),
                stop=True,
            )
        # row sums over sk (partitions + tiles) via ones-matmul
        rs = psum_pool.tile([P, 512], F32, tag="st", name=f"rs_{label}_{c}")
        for j in range(mt):
            nc.tensor.matmul(
                rs[0:1, :fw],
                ones_r[:],
                p_tiles[j][:, :fw],
                start=(j == 0),
                stop=(j == mt - 1),
            )
        recip = io.tile([1, 512], F32R, tag="recip", name=f"recip_{label}_{c}", bufs=2)
        with nc.allow_low_precision(reason="f32r recip feeds f32r bcast matmul"):
            nc.vector.reciprocal(recip[:, :fw], rs[0:1, :fw])
        if recip_col is None:
            # broadcast recip across partitions via K=1 f32r matmul
            bc = psum_pool.tile([P, 512], F32, tag="st", name=f"bc_{label}_{c}")
            nc.tensor.matmul(bc[:, :fw], ones_row_r[:], recip[:, :fw], start=True, stop=True)
            rbc = io.tile([P, 512], F32, tag="rbc", name=f"rbc_{label}_{c}", bufs=4)
            nc.vector.tensor_copy(out=rbc[:, :fw], in_=bc[:, :fw])
            rbcs.append(rbc[:, :fw])
        else:
            # column layout recip_col[p, a] = 1/rowsum[sq = off + a*P + p]
            # via a DRAM bounce (free->partition reshuffles need DMA via DRAM)
            scr = dram_pool.tile([1, 512], F32, tag="rscr", name=f"rscr_{label}_{c}", bufs=2)
            nc.sync.dma_start(scr[:, :fw], recip[:, :fw].bitcast(F32))
            nfull = fw // P
            rem = fw - nfull * P
            if nfull:
                nc.sync.dma_start(
                    recip_col[:, off // P : off // P + nfull],
                    scr[0, : nfull * P].rearrange("(a p) -> p a", p=P),
                )
            if rem:
                nc.sync.dma_start(
                    recip_col[0:rem, off // P + nfull : off // P + nfull + 1],
                    scr[0, nfull * P : fw].rearrange("(a p) -> p a", p=rem),
                )
        # UNNORMALIZED copyback (releases psum_o banks immediately)
        for d in range(KT):
            nc.vector.tensor_copy(out=outT[:, d, sq], in_=pos[d][:, :fw])
    return rbcs


def _attention_nat(nc, io, psum_att, psum_main, qT, kT, vn, out_d, rowsum_d,
                   colb, rm_scaled, ones_r, label, mt, qch):
    """Natural-output attention: out_d[sq, :] = UNNORMALIZED numerator
    p @ vn (rows on partitions), rowsum_d[0, sq] = per-query exp-sums.
    Normalization (and the final bias) happen on the host.

    vn here is the folded v' = t @ (Wcv @ Wffn) + bcv @ Wffn, so this fuses
    attention-2's output accumulation with the reference's trailing ffn.
    """
    for (off, fw) in qch:
        sq = slice(off, off + fw)
        subs = []
        q0 = 0
        while q0 < fw:
            subs.append((q0, min(P, fw - q0)))
            q0 += P
        pos = [
            psum_att.tile([P, C], F32, tag="pon", name=f"pon_{label}_{off}_{si}")
            for si in range(len(subs))
        ]
        p_tiles = []
        prev = None
        for j in range(mt):
            st = psum_main.tile([P, 512], F32, tag="st", name=f"st_{label}_{off}_{j}")
            for a in range(KT):
                nc.tensor.matmul(
                    st[:, :fw],
                    kT[:, a, j * P : (j + 1) * P],
                    qT[:, a, sq],
                    start=(a == 0),
                    stop=(a == KT - 1),
                )
            mk = io.tile([P, 512], F32, tag="mk", name=f"mk_{label}_{off}_{j}", bufs=2)
            nc.vector.scalar_tensor_tensor(
                out=mk[:, :fw],
                in0=st[:, :fw],
                scalar=colb[:, j : j + 1],
                in1=rm_scaled[:, sq],
                op0=AL.add,
                op1=AL.mult,
            )
            pj = io.tile([P, 512], F32R, tag="pp", name=f"p_{label}_{off}_{j}", bufs=mt + 2)
            nc.scalar.activation(pj[:, :fw], mk[:, :fw], AF.Exp)
            p_tiles.append(pj)
            if prev is not None:
                jj, pprev = prev
                for si, (qo, qw) in enumerate(subs):
                    for (coff, cw) in [(0, 512), (512, 256)]:
                        nc.tensor.matmul(
                            pos[si][0:qw, coff : coff + cw],
                            pprev[:, qo : qo + qw],
                            vn[:, jj, coff : coff + cw],
                            start=(jj == 0),
                            stop=False,
                        )
            prev = (j, pj)
        jj, pprev = prev
        for si, (qo, qw) in enumerate(subs):
            for (coff, cw) in [(0, 512), (512, 256)]:
                nc.tensor.matmul(
                    pos[si][0:qw, coff : coff + cw],
                    pprev[:, qo : qo + qw],
                    vn[:, jj, coff : coff + cw],
                    start=(jj == 0),
                    stop=True,
                )
        # row sums over sk via ones-matmul, straight to DRAM (host divides)
        rs = psum_main.tile([P, 512], F32, tag="st", name=f"rs_{label}_{off}")
        for j in range(mt):
            nc.tensor.matmul(
                rs[0:1, :fw],
                ones_r[:],
                p_tiles[j][:, :fw],
                start=(j == 0),
                stop=(j == mt - 1),
            )
        rsb = io.tile([1, 512], F32, tag="rsb", name=f"rsb_{label}_{off}", bufs=2)
        nc.vector.tensor_copy(out=rsb[:, :fw], in_=rs[0:1, :fw])
        nc.sync.dma_start(rowsum_d[0:1, off : off + fw], rsb[:, :fw])
        for si, (qo, qw) in enumerate(subs):
            fin = io.tile([P, C], F32, tag="fin", name=f"fin_{label}_{off}_{si}", bufs=2)
            nc.vector.tensor_copy(out=fin[0:qw, :], in_=pos[si][0:qw, :])
            nc.sync.dma_start(out_d[off + qo : off + qo + qw, :], fin[0:qw, :])


def _transpose_in(nc, io, psum_tr, src_dram, dst, ident, tag, mt):
    """dst [P, KT, MV] (f32r) = src^T, via PE 128x128 transposes."""
    for i in range(mt):
        xin = io.tile([P, C], F32R, tag="xin", name=f"xin_{tag}_{i}", bufs=3)
        nc.sync.dma_start(xin[:], src_dram[i * P : (i + 1) * P, :])
        for a in range(KT):
            tr = psum_tr.tile([P, P], F32R, tag="tr", name=f"tr_{tag}_{i}_{a}")
            nc.tensor.transpose(tr[:], xin[:, a * P : (a + 1) * P], ident[:])
            nc.any.tensor_copy(out=dst[:, a, i * P : (i + 1) * P], in_=tr[:])


def build_nc(n_iters=1, mv=MV_DEFAULT, mq=None):
    """mv: key/seq extent (multiple of 128); mq: query extent (multiple of
    32, <= mv) — query rows past mq are pad and never computed."""
    if mq is None:
        mq = MQ_DEFAULT if mv == MV_DEFAULT else mv
    mt = mv // P
    qch = _qchunks(mq)
    kch = _qchunks(mv)

    nc = bacc.Bacc(trn_type="TRN2", target_bir_lowering=False, debug=False)

    x_d = nc.dram_tensor("x", [mv, C], F32R, kind="ExternalInput").ap()
    t_d = nc.dram_tensor("t", [mv, C], F32R, kind="ExternalInput").ap()
    mask_d = nc.dram_tensor("mask", [1, mv], F32, kind="ExternalInput").ap()
    g1_d = nc.dram_tensor("G1", [C, C], F32R, kind="ExternalInput").ap()
    g1b_d = nc.dram_tensor("g1b", [1, C], F32, kind="ExternalInput").ap()
    wv_d = nc.dram_tensor("Wv", [C, C], F32R, kind="ExternalInput").ap()
    bv_d = nc.dram_tensor("bv", [1, C], F32, kind="ExternalInput").ap()
    g2_d = nc.dram_tensor("G2", [C, C], F32R, kind="ExternalInput").ap()
    g2b_d = nc.dram_tensor("g2b", [1, C], F32, kind="ExternalInput").ap()
    wcv_d = nc.dram_tensor("Wcv", [C, C], F32R, kind="ExternalInput").ap()
    bcv_d = nc.dram_tensor("bcv", [1, C], F32, kind="ExternalInput").ap()
    wffn_d = nc.dram_tensor("Wffn", [C, C], F32R, kind="ExternalInput").ap()
    bffn_d = nc.dram_tensor("bffn", [1, C], F32, kind="ExternalInput").ap()
    out_d = nc.dram_tensor("out", [mv, C], F32, kind="ExternalOutput").ap()

    g1_t = g1_d.rearrange("(a p) n -> p a n", p=P)  # [P, KT, C]
    wv_t = wv_d.rearrange("(a p) n -> p a n", p=P)
    g2_t = g2_d.rearrange("(a p) n -> p a n", p=P)
    wcv_t = wcv_d.rearrange("(a p) n -> p a n", p=P)
    wffn_t = wffn_d.rearrange("(a p) n -> p a n", p=P)

    with tile.TileContext(nc) as tc, ExitStack() as ctx:
        const = ctx.enter_context(tc.tile_pool(name="const", bufs=1))
        acts = ctx.enter_context(tc.tile_pool(name="acts", bufs=1))
        wpool = ctx.enter_context(tc.tile_pool(name="wpool", bufs=1))
        io = ctx.enter_context(tc.tile_pool(name="io", bufs=1))
        psum_main = ctx.enter_context(tc.tile_pool(name="psum_main", bufs=2, space="PSUM"))

        # ---- constants ----
        ident32 = const.tile([P, P], F32, tag="ident32", name="ident32")
        make_identity(nc, ident32[:])
        ident = const.tile([P, P], F32R, tag="ident", name="ident")
        nc.vector.tensor_copy(out=ident[:], in_=ident32[:])

        mask_t = const.tile([P, mt], F32, tag="mask_t", name="mask_t")
        nc.sync.dma_start(mask_t[:], mask_d[0].rearrange("(a p) -> p a", p=P))
        colb = const.tile([P, mt], F32, tag="colb", name="colb")
        nc.vector.tensor_scalar(
            colb[:], mask_t[:], 10000.0 / SCALE, -10000.0 / SCALE, AL.mult, AL.add
        )

        rm_scaled = const.tile([P, mv], F32, tag="rm_scaled", name="rm_scaled")
        nc.sync.dma_start(rm_scaled[:], mask_d.partition_broadcast(P))
        nc.vector.tensor_scalar_mul(rm_scaled[:], rm_scaled[:], SCALE)

        ones32 = const.tile([P, 1], F32, tag="ones32", name="ones32")
        nc.gpsimd.memset(ones32[:], 1.0)
        ones_r = const.tile([P, 1], F32R, tag="ones_r", name="ones_r")
        nc.vector.tensor_copy(out=ones_r[:], in_=ones32[:])
        ones_row32 = const.tile([1, P], F32, tag="ones_row32", name="ones_row32")
        nc.gpsimd.memset(ones_row32[:], 1.0)
        ones_row_r = const.tile([1, P], F32R, tag="ones_row_r", name="ones_row_r")
        nc.vector.tensor_copy(out=ones_row_r[:], in_=ones_row32[:])

        # per-partition bias columns (d on partitions)
        g1b_col = const.tile([P, KT], F32, tag="g1b_col", name="g1b_col")
        nc.sync.dma_start(g1b_col[:], g1b_d[0, :].rearrange("(a p) -> p a", p=P))
        g2b_col = const.tile([P, KT], F32, tag="g2b_col", name="g2b_col")
        nc.sync.dma_start(g2b_col[:], g2b_d[0, :].rearrange("(a p) -> p a", p=P))

        # ---- big activation tensors ----
        for _it in range(n_iters):
            _body_iter(nc, tc, ctx, acts, wpool, io, const, psum_main,
                       x_d, t_d, g1_t, wv_t, g2_t, wcv_t, wffn_t,
                       bv_d, bcv_d, bffn_d, out_d,
                       ident, colb, rm_scaled, ones_r, ones_row_r,
                       g1b_col, g2b_col, _it, mt, qch, kch, mv, mq)

    nc.compile()
    return nc


def _body_iter(nc, tc, ctx, acts, wpool, io, const, psum_main,
               x_d, t_d, g1_t, wv_t, g2_t, wcv_t, wffn_t,
               bv_d, bcv_d, bffn_d, out_d,
               ident, colb, rm_scaled, ones_r, ones_row_r,
               g1b_col, g2b_col, it, mt, qch, kch, mv, mq):
    if True:
        # scores are computed via the folded form s = (x@G + Wk@bq) . x
        # (softmax is invariant to the dropped per-query terms), so the
        # attention KEYS are the raw transposed inputs xT / tT and the k/ck
        # projections never happen.
        xT = acts.tile([P, KT, mv], F32R, tag="xT", name="xT")  # x^T
        qT = acts.tile([P, KT, mv], F32R, tag="qT", name="qT")  # g1^T
        vn = acts.tile([P, mt, C], F32R, tag="vn", name="vn")
        o1T = acts.tile([P, KT, mv], F32R, tag="oT", name="o1T")

        # ---- phase A: transpose x ----
        psum_tr = tc.alloc_tile_pool(name="psum_tr", bufs=6, space="PSUM")
        _transpose_in(nc, io, psum_tr, x_d, xT, ident, f"x{it}", mt)

        # ---- phase B: g1 + v projections ----
        bias_bc = wpool.tile([P, C], F32, tag="bbc", name="vbias_bc")
        nc.sync.dma_start(bias_bc[:], bv_d[0:1, :].partition_broadcast(P))

        for d in range(KT):
            w = wpool.tile([P, KT, P], F32R, tag="ws", name=f"wsg1_{d}", bufs=3)
            nc.sync.dma_start(w[:], g1_t[:, :, d * P : (d + 1) * P])
            for (off, fw) in qch:
                ps = psum_main.tile([P, 512], F32, tag="st", name=f"ps_g1_{d}_{off}")
                for a in range(KT):
                    nc.tensor.matmul(
                        ps[:, :fw],
                        w[:, a, :],
                        xT[:, a, off : off + fw],
                        start=(a == 0),
                        stop=(a == KT - 1),
                    )
                nc.any.tensor_scalar_add(
                    qT[:, d, off : off + fw], ps[:, :fw], g1b_col[:, d : d + 1]
                )

        vw = wpool.tile([P, KT, C], F32R, tag="vw", name="vw_v")
        nc.sync.dma_start(vw[:], wv_t[:])
        _proj_natural(nc, xT, vw, vn, bias_bc, psum_main, mt)

        # ---- phase A2: transpose t (kT slot; live through attn2) ----
        tT = acts.tile([P, KT, mv], F32R, tag="kT", name="tT")
        _transpose_in(nc, io, psum_tr, t_d, tT, ident, f"t{it}", mt)
        psum_tr.release()

        psum_att = tc.alloc_tile_pool(name="psum_att", bufs=6, space="PSUM")

        # ---- phase C/D: attention 1 (keys = xT) ----
        class _AttPsum:
            def tile(self, shape, dtype, tag, name):
                pool = psum_att if tag == "po" else psum_main
                return pool.tile(shape, dtype, tag=tag, name=name)

        att_psum = _AttPsum()
        rbcs1 = _attention(
            nc, io, att_psum, qT, xT, vn, o1T, colb, rm_scaled,
            ones_r, ones_row_r, "a1", mt, qch,
        )

        # ---- phase E: g2 projection (into qT slot) ----
        g2T = acts.tile([P, KT, mv], F32R, tag="qT", name="g2T")
        wqs = wpool.tile([P, KT, C], F32R, tag="vw", name="g2_sb")
        nc.sync.dma_start(wqs[:], g2_t[:])
        for d in range(KT):
            for ci, (off, fw) in enumerate(qch):
                ps = psum_main.tile([P, 512], F32, tag="st", name=f"ps_g2_{d}_{off}")
                for a in range(KT):
                    nc.tensor.matmul(
                        ps[:, :fw],
                        wqs[:, a, d * P : (d + 1) * P],
                        o1T[:, a, off : off + fw],
                        start=(a == 0),
                        stop=(a == KT - 1),
                    )
                dst = g2T[:, d, off : off + fw]
                nc.any.tensor_mul(out=dst, in0=ps[:, :fw], in1=rbcs1[ci])
                nc.any.tensor_scalar_add(dst, dst, g2b_col[:, d : d + 1])

        # ---- phase F: cv projection from t (into vn slot) ----
        cvn = acts.tile([P, mt, C], F32R, tag="vn", name="cvn")
        cvw = wpool.tile([P, KT, C], F32R, tag="vw", name="vw_cv")
        nc.sync.dma_start(cvw[:], wcv_t[:])
        cv_bias_bc = wpool.tile([P, C], F32, tag="bbc", name="cvbias_bc")
        nc.sync.dma_start(cv_bias_bc[:], bcv_d[0:1, :].partition_broadcast(P))
        _proj_natural(nc, tT, cvw, cvn, cv_bias_bc, psum_main, mt)

        # ---- phase G: attention 2 (keys = tT; out2T into xT slot) ----
        o2T = acts.tile([P, KT, mv], F32R, tag="xT", name="o2T")
        recip2_col = io.tile([P, mt], F32, tag="recip2_col", name="recip2_col", bufs=2)
        if mq < mv:
            # pad query cols are never computed but ARE read by the ffn's
            # last seq tile (its output rows land in discarded pad rows) —
            # zero them so those reads stay finite.
            for a in range(KT):
                nc.gpsimd.memset(o2T[:, a, mq:mv].bitcast(F32), 0.0)
            nc.gpsimd.memset(recip2_col[:], 1.0)
        dram_pool = tc.alloc_tile_pool(name="dram_scr", bufs=1, space="DRAM")
        # small chunk first: its recip DRAM-bounce (needed by the ffn's
        # last seq tile) overlaps the big chunk's compute
        _attention(
            nc, io, att_psum, g2T, tT, cvn, o2T, colb, rm_scaled,
            ones_r, ones_row_r, "a2", mt, qch[::-1],
            recip_col=recip2_col, dram_pool=dram_pool,
        )
        dram_pool.release()

        # ---- phase H: ffn ----
        wfs = wpool.tile([P, KT, C], F32R, tag="vw", name="wffn_sb")
        nc.sync.dma_start(wfs[:], wffn_t[:])
        ffn_bias_bc = wpool.tile([P, C], F32, tag="bbc", name="ffnbias_bc")
        nc.sync.dma_start(ffn_bias_bc[:], bffn_d[0:1, :].partition_broadcast(P))
        chunks = [(0, 512), (512, 256)]
        for i in range(mt):
            pss = []
            for (off, w) in chunks:
                ps = psum_main.tile([P, 512], F32, tag="st", name=f"ps_f_{i}_{off}")
                for a in range(KT):
                    nc.tensor.matmul(
                        ps[:, :w],
                        o2T[:, a, i * P : (i + 1) * P],
                        wfs[:, a, off : off + w],
                        start=(a == 0),
                        stop=(a == KT - 1),
                    )
                pss.append(ps)
            fin = io.tile([P, C], F32, tag="fin", name=f"fin_{i}", bufs=2)
            for (off, w), ps in zip(chunks, pss):
                nc.vector.scalar_tensor_tensor(
                    out=fin[:, off : off + w],
                    in0=ps[:, :w],
                    scalar=recip2_col[:, i : i + 1],
                    in1=ffn_bias_bc[:, off : off + w],
                    op0=AL.mult,
                    op1=AL.add,
                )
            nc.sync.dma_start(out_d[i * P : (i + 1) * P, :], fin[:])

        psum_att.release()


_NC_CACHE = {}


def _get_nc(dims=(MV_DEFAULT, MQ_DEFAULT)):
    if dims not in _NC_CACHE:
        _NC_CACHE[dims] = build_nc(mv=dims[0], mq=dims[1])
    return _NC_CACHE[dims]


def prep_inputs(layout_x, text_x, mask, Wqkv, bqkv, Wq, bq, Wkv, bkv, Wffn, bffn):
    """Host-side valid-row compaction.

    Returns (mv, in_maps, idxs, fill) where in_maps feeds the device kernel
    (compacted to mv rows per core), idxs[b] are the valid row indices, and
    fill[b] is the output vector for every masked row of batch b.
    """
    layout_x = np.ascontiguousarray(np.asarray(layout_x, dtype=np.float32))
    text_x = np.ascontiguousarray(np.asarray(text_x, dtype=np.float32))
    mask = np.ascontiguousarray(np.asarray(mask, dtype=np.float32))
    Wqkv = np.ascontiguousarray(np.asarray(Wqkv, dtype=np.float32))
    bqkv = np.ascontiguousarray(np.asarray(bqkv, dtype=np.float32)).reshape(1, 3 * C)
    Wq = np.ascontiguousarray(np.asarray(Wq, dtype=np.float32))
    bq = np.ascontiguousarray(np.asarray(bq, dtype=np.float32)).reshape(1, C)
    Wkv = np.ascontiguousarray(np.asarray(Wkv, dtype=np.float32))
    bkv = np.ascontiguousarray(np.asarray(bkv, dtype=np.float32)).reshape(1, 2 * C)
    Wffn = np.ascontiguousarray(np.asarray(Wffn, dtype=np.float32))
    bffn = np.ascontiguousarray(np.asarray(bffn, dtype=np.float32)).reshape(1, C)

    B = layout_x.shape[0]
    assert B == N_CORES

    idxs = [np.nonzero(mask[b] != 0)[0] for b in range(B)]
    nvs = [len(ix) for ix in idxs]
    mv = min(M, max(P, -(-max(nvs) // P) * P))
    mq = min(mv, max(P, -(-max(nvs) // 32) * 32))

    # masked-row output: uniform attn2 over ALL cv rows, then ffn
    tx_mean = text_x.astype(np.float64).mean(axis=1)  # [B, C]
    cv_mean = tx_mean @ Wkv[:, C:].astype(np.float64) + bkv[0, C:].astype(np.float64)
    fill = (cv_mean @ Wffn.astype(np.float64) + bffn[0].astype(np.float64)).astype(
        np.float32
    )  # [B, C]

    # fold q/k projections: s = (x@Wq + bq).(x@Wk + bk) == (x@G + Wk@bq).x
    # up to per-query additive constants that cancel in softmax.
    G1 = np.ascontiguousarray(Wqkv[:, :C] @ Wqkv[:, C : 2 * C].T)
    g1b = (Wqkv[:, C : 2 * C] @ bqkv[0, :C]).reshape(1, C)
    G2 = np.ascontiguousarray(Wq @ Wkv[:, :C].T)
    g2b = (Wkv[:, :C] @ bq[0]).reshape(1, C)
    Wv = np.ascontiguousarray(Wqkv[:, 2 * C :])
    bv = np.ascontiguousarray(bqkv[:, 2 * C :])
    Wcv = np.ascontiguousarray(Wkv[:, C:])
    bcv = np.ascontiguousarray(bkv[:, C:])

    in_maps = []
    for b in range(B):
        nv = nvs[b]
        x_c = np.zeros((mv, C), np.float32)
        x_c[:nv] = layout_x[b][idxs[b]]
        t_c = np.zeros((mv, C), np.float32)
        t_c[:nv] = text_x[b][idxs[b]]
        m_c = np.zeros((1, mv), np.float32)
        m_c[0, :nv] = 1.0
        in_maps.append(
            {
                "x": x_c,
                "t": t_c,
                "mask": m_c,
                "G1": G1,
                "g1b": g1b,
                "Wv": Wv,
                "bv": bv,
                "G2": G2,
                "g2b": g2b,
                "Wcv": Wcv,
                "bcv": bcv,
                "Wffn": Wffn,
                "bffn": bffn,
            }
        )
    return (mv, mq), in_maps, idxs, fill


def finish_output(dev_outs, idxs, fill):
    """Scatter compacted device outputs back to the full [B, M, C] shape."""
    B = len(idxs)
    out = np.empty((B, M, C), np.float32)
    for b in range(B):
        out[b, :, :] = fill[b]
        out[b, idxs[b], :] = dev_outs[b][: len(idxs[b])]
    return out


def kernel(
    layout_x, text_x, mask, Wqkv, bqkv, Wq, bq, Wkv, bkv, Wffn, bffn
):
    dims, in_maps, idxs, fill = prep_inputs(
        layout_x, text_x, mask, Wqkv, bqkv, Wq, bq, Wkv, bkv, Wffn, bffn
    )
    nc = _get_nc(dims)
    res = run_bass_kernel_spmd(nc, in_maps, core_ids=list(range(N_CORES)))
    return finish_output(
        [res.results[b]["out"] for b in range(N_CORES)], idxs, fill
    )


# revision 32
# speedup vs baseline: 2.8212x; 1.5481x over previous
"""Trainium2 Bass kernel for nn_Attention_kv (dense transformer block).

Sharding: data-parallel over batch B=8 across the 8 NeuronCores — one batch
element per core, no collectives (host scatters inputs / stacks outputs).

Valid-row compaction (host side): ~50% of sequence positions have mask==0.
Masked QUERY rows of both attentions get uniform attention over ALL keys, so
every masked row of the final output equals one per-batch vector
  fill[b] = (mean_M(text_x[b]) @ Wkv[:,C:] + bkv[C:]) @ Wffn + bffn
computable on the host in microseconds.  Valid rows never depend on masked
rows: an invalid key's attention weight is exp(s*scale - 10000) == 0.0 in
fp32 (hard underflow), identical to the reference's jnp.where(-10000) +
softmax.  So the device only sees the gathered valid rows, padded to a
multiple of 128 (634 max over batches -> MV=640 for the graded inputs), with
a recomputed 1/0 mask for the pad tail; the host scatters device rows back
into the valid positions and broadcast-fills masked rows with fill[b].
This removes ~48% of all PE work (projections scale by MV/M, attention by
(MV/M)^2) and is exact up to fp32 reassociation.

Projection folding (host side, all exact): softmax is invariant to
per-query additive score constants, so
  scores1 = (x@Wq1 + bq1).(x@Wk1 + bk1)  ==  (x@G1 + Wk1@bq1) . x
with G1 = Wq1@Wk1^T — the k-projection vanishes and attention keys are the
raw transposed input.  Likewise for attention 2 (G2 = Wq@Wkc^T, keys = t).
Further, per-row scalars commute with matmuls, so
  g2 = ((p1@v)*recip)@G2 + b  ==  (p1@(v@G2))*recip + b
means attn1 accumulates with the folded v' = x@(Wv@G2) and its output IS
raw g2 (the g2 projection becomes a vector normalize pass), and since
normalized attention rows sum to 1,
  out = (p2@cv)@Wffn + bffn  ==  p2@(cv@Wffn) + bffn
means attn2 accumulates with cv' = t@(Wcv@Wffn) and its raw output IS the
final numerator — the ffn phase vanishes; the host divides by the exported
row sums and adds bffn.

Per-core pipeline (seq MV, dim C=768):
  x^T via PE 128x128 transposes -> g1 projection (transposed [d, seq]) and
  v' projection (natural) -> t^T transposes
  -> attn1: scores TRANSPOSED S^T[sk, sq] with keys = x^T; max-free masked
     softmax (additive -10000 key mask + multiplicative query-mask zeroing);
     row sums via PE ones-matmul; raw-g2^T accumulated across 6 PSUM banks
     flash-style; normalization deferred into a vector pass
  -> cv' projection (from t) -> attn2 with keys = t^T, output accumulated
     NATURALLY per 128-query sub-tile and DMA'd raw; row sums exported.

All matmuls run in float32r (TF32-like PE datapath, 1 cycle/row — measured
numerically identical to the fp32 4-cycle/row path on this hardware).
"""

import sys

sys.path.insert(0, "/opt/trn_rl_repo")

from contextlib import ExitStack

import numpy as np

import concourse.bass as bass
import concourse.mybir as mybir
import concourse.tile as tile
from concourse import bacc
from concourse.bass_utils import run_bass_kernel_spmd
from concourse.masks import make_identity

P = 128
M = 1024  # full sequence length per batch element (host side)
C = 768  # model dim
KT = C // P  # 6 contraction tiles
MV_DEFAULT = 640  # compacted device seq len for the graded inputs
MQ_DEFAULT = 544  # compacted query extent (max 534 valid rows per batch)
SCALE = float(C) ** -0.5
NEG = -10000.0

F32 = mybir.dt.float32
F32R = mybir.dt.float32r
AL = mybir.AluOpType
AF = mybir.ActivationFunctionType

N_CORES = 8


def _qchunks(mv):
    """Split the device seq dim into free-dim chunks of <= 512 (one PSUM
    bank of fp32)."""
    out = []
    off = 0
    while off < mv:
        w = min(512, mv - off)
        out.append((off, w))
        off += w
    return out


def _proj_natural(nc, lhs_src, w_rhs, dst, bias_bc, psum_pool, mt):
    """dst[:, i, :] (shape [P, mt, C]) = src @ W + bias.

    lhs_src: AP [P, KT, MV] (x^T layout, f32r) -- lhsT tiles [P, 128]
    w_rhs: AP [P, KT, C] (weight, f32r) -- rhs tiles [P, chunk]
    bias_bc: AP [P, C] broadcast bias or None
    """
    chunks = [(0, 512), (512, 256)]
    for i in range(mt):
        pss = []
        for (off, w) in chunks:
            ps = psum_pool.tile([P, 512], F32, tag="st", name=f"ps_v_{i}_{off}")
            for a in range(KT):
                nc.tensor.matmul(
                    ps[:, :w],
                    lhs_src[:, a, i * P : (i + 1) * P],
                    w_rhs[:, a, off : off + w],
                    start=(a == 0),
                    stop=(a == KT - 1),
                )
            pss.append(ps)
        for (off, w), ps in zip(chunks, pss):
            if bias_bc is not None:
                nc.any.tensor_add(
                    out=dst[:, i, off : off + w],
                    in0=ps[:, :w],
                    in1=bias_bc[:, off : off + w],
                )
            else:
                nc.any.tensor_copy(out=dst[:, i, off : off + w], in_=ps[:, :w])


def _attention(nc, io, psum_pool, qT, kT, vn, outT, colb, rm_scaled,
               ones_r, ones_row_r, label, mt, qch,
               recip_col=None, dram_pool=None):
    # qch are the QUERY chunks and may stop short of the key extent
    # (mt*P): pad queries past max-valid-rows are never consumed.
    """outT[:, d, :] = (UNNORMALIZED attn numerator)^T, [P, KT, MV] f32r.

    Normalization is deferred to the consumer: returns per-chunk rbc
    broadcast tile slices [P, fw] (1/rowsum along free sq) unless recip_col
    is given, in which case recip values are instead written into
    recip_col ([P, mt] column layout) and no bcast is made.

    qT, kT: [P, KT, MV] f32r (d on partitions); vn: [P, mt, C] f32r.
    colb: [P, mt] fp32 = (mask-1)*10000/scale along sk partitions.
    rm_scaled: [P, MV] fp32 = mask*scale broadcast (varies along free sq).
    """
    rbcs = []
    for c, (off, fw) in enumerate(qch):
        sq = slice(off, off + fw)
        # out^T accumulators: 6 banks
        pos = [
            psum_pool.tile([P, 512], F32, tag="po", name=f"po_{label}_{c}_{d}")
            for d in range(KT)
        ]
        p_tiles = []
        prev = None  # (j, p_j) pending out^T matmuls
        for j in range(mt):
            st = psum_pool.tile([P, 512], F32, tag="st", name=f"st_{label}_{c}_{j}")
            for a in range(KT):
                nc.tensor.matmul(
                    st[:, :fw],
                    kT[:, a, j * P : (j + 1) * P],
                    qT[:, a, sq],
                    start=(a == 0),
                    stop=(a == KT - 1),
                )
            # masked = (S^T + colb_j) * rm_scaled ; exp
            mk = io.tile([P, 512], F32, tag="mk", name=f"mk_{label}_{c}_{j}", bufs=2)
            nc.vector.scalar_tensor_tensor(
                out=mk[:, :fw],
                in0=st[:, :fw],
                scalar=colb[:, j : j + 1],
                in1=rm_scaled[:, sq],
                op0=AL.add,
                op1=AL.mult,
            )
            pj = io.tile([P, 512], F32R, tag="pp", name=f"p_{label}_{c}_{j}", bufs=mt + 2)
            nc.scalar.activation(pj[:, :fw], mk[:, :fw], AF.Exp)
            p_tiles.append(pj)
            if prev is not None:
                jj, pprev = prev
                for d in range(KT):
                    nc.tensor.matmul(
                        pos[d][:, :fw],
                        vn[:, jj, d * P : (d + 1) * P],
                        pprev[:, :fw],
                        start=(jj == 0),
                        stop=False,
                    )
            prev = (j, pj)
        jj, pprev = prev
        for d in range(KT):
            nc.tensor.matmul(
                pos[d][:, :fw],
                vn[:, jj, d * P : (d + 1) * P],
                pprev[:, :fw],
                start=(jj == 0),
                stop=True,
            )
        # row sums over sk (partitions + tiles) via ones-matmul
        rs = psum_pool.tile([P, 512], F32, tag="st", name=f"rs_{label}_{c}")
        for j in range(mt):
            nc.tensor.matmul(
                rs[0:1, :fw],
                ones_r[:],
                p_tiles[j][:, :fw],
                start=(j == 0),
                stop=(j == mt - 1),
            )
        recip = io.tile([1, 512], F32R, tag="recip", name=f"recip_{label}_{c}", bufs=2)
        with nc.allow_low_precision(reason="f32r recip feeds f32r bcast matmul"):
            nc.vector.reciprocal(recip[:, :fw], rs[0:1, :fw])
        if recip_col is None:
            # broadcast recip across partitions via K=1 f32r matmul
            bc = psum_pool.tile([P, 512], F32, tag="st", name=f"bc_{label}_{c}")
            nc.tensor.matmul(bc[:, :fw], ones_row_r[:], recip[:, :fw], start=True, stop=True)
            rbc = io.tile([P, 512], F32, tag="rbc", name=f"rbc_{label}_{c}", bufs=4)
            nc.vector.tensor_copy(out=rbc[:, :fw], in_=bc[:, :fw])
            rbcs.append(rbc[:, :fw])
        else:
            # column layout recip_col[p, a] = 1/rowsum[sq = off + a*P + p]
            # via a DRAM bounce (free->partition reshuffles need DMA via DRAM)
            scr = dram_pool.tile([1, 512], F32, tag="rscr", name=f"rscr_{label}_{c}", bufs=2)
            nc.sync.dma_start(scr[:, :fw], recip[:, :fw].bitcast(F32))
            nfull = fw // P
            rem = fw - nfull * P
            if nfull:
                nc.sync.dma_start(
                    recip_col[:, off // P : off // P + nfull],
                    scr[0, : nfull * P].rearrange("(a p) -> p a", p=P),
                )
            if rem:
                nc.sync.dma_start(
                    recip_col[0:rem, off // P + nfull : off // P + nfull + 1],
                    scr[0, nfull * P : fw].rearrange("(a p) -> p a", p=rem),
                )
        # UNNORMALIZED copyback (releases psum_o banks immediately)
        for d in range(KT):
            nc.vector.tensor_copy(out=outT[:, d, sq], in_=pos[d][:, :fw])
    return rbcs


def _attention_nat(nc, io, psum_att, psum_main, qT, kT, vn, out_d, rowsum_d,
                   colb, rm_scaled, ones_r, label, mt, qch):
    """Natural-output attention: out_d[sq, :] = UNNORMALIZED numerator
    p @ vn (rows on partitions), rowsum_d[0, sq] = per-query exp-sums.
    Normalization (and the final bias) happen on the host.

    vn here is the folded v' = t @ (Wcv @ Wffn) + bcv @ Wffn, so this fuses
    attention-2's output accumulation with the reference's trailing ffn.
    """
    for (off, fw) in qch:
        sq = slice(off, off + fw)
        subs = []
        q0 = 0
        while q0 < fw:
            subs.append((q0, min(P, fw - q0)))
            q0 += P
        # all score/exp tiles of the chunk first (they all feed rowsum)
        p_tiles = []
        for j in range(mt):
            st = psum_main.tile([P, 512], F32, tag="st", name=f"st_{label}_{off}_{j}")
            for a in range(KT):
                nc.tensor.matmul(
                    st[:, :fw],
                    kT[:, a, j * P : (j + 1) * P],
                    qT[:, a, sq],
                    start=(a == 0),
                    stop=(a == KT - 1),
                )
            mk = io.tile([P, 512], F32, tag="mk", name=f"mk_{label}_{off}_{j}", bufs=2)
            nc.vector.scalar_tensor_tensor(
                out=mk[:, :fw],
                in0=st[:, :fw],
                scalar=colb[:, j : j + 1],
                in1=rm_scaled[:, sq],
                op0=AL.add,
                op1=AL.mult,
            )
            pj = io.tile([P, 512], F32R, tag="pp", name=f"p_{label}_{off}_{j}", bufs=mt + 2)
            nc.scalar.activation(pj[:, :fw], mk[:, :fw], AF.Exp)
            p_tiles.append(pj)
        # row sums over sk via ones-matmul, straight to DRAM (host divides)
        rs = psum_main.tile([P, 512], F32, tag="st", name=f"rs_{label}_{off}")
        for j in range(mt):
            nc.tensor.matmul(
                rs[0:1, :fw],
                ones_r[:],
                p_tiles[j][:, :fw],
                start=(j == 0),
                stop=(j == mt - 1),
            )
        rsb = io.tile([1, 512], F32, tag="rsb", name=f"rsb_{label}_{off}", bufs=2)
        nc.vector.tensor_copy(out=rsb[:, :fw], in_=rs[0:1, :fw])
        nc.sync.dma_start(rowsum_d[0:1, off : off + fw], rsb[:, :fw])
        # output accumulation per 128-query sub-tile (p^T slices as lhsT),
        # [P,512]+[P,256] psum pairs to stay within PSUM banks
        for si, (qo, qw) in enumerate(subs):
            pa = psum_att.tile([P, 512], F32, tag="pona", name=f"pona_{label}_{off}_{si}")
            pb = psum_att.tile([P, 256], F32, tag="ponb", name=f"ponb_{label}_{off}_{si}")
            for jj in range(mt):
                nc.tensor.matmul(
                    pa[0:qw, :],
                    p_tiles[jj][:, qo : qo + qw],
                    vn[:, jj, 0:512],
                    start=(jj == 0),
                    stop=(jj == mt - 1),
                )
                nc.tensor.matmul(
                    pb[0:qw, :],
                    p_tiles[jj][:, qo : qo + qw],
                    vn[:, jj, 512:C],
                    start=(jj == 0),
                    stop=(jj == mt - 1),
                )
            fin = io.tile([P, C], F32, tag="fin", name=f"fin_{label}_{off}_{si}", bufs=2)
            nc.vector.tensor_copy(out=fin[0:qw, 0:512], in_=pa[0:qw, :])
            nc.vector.tensor_copy(out=fin[0:qw, 512:C], in_=pb[0:qw, :])
            nc.sync.dma_start(out_d[off + qo : off + qo + qw, :], fin[0:qw, :])


def _transpose_in(nc, io, psum_tr, src_dram, dst, ident, tag, mt):
    """dst [P, KT, MV] (f32r) = src^T, via PE 128x128 transposes."""
    for i in range(mt):
        xin = io.tile([P, C], F32R, tag="xin", name=f"xin_{tag}_{i}", bufs=3)
        nc.sync.dma_start(xin[:], src_dram[i * P : (i + 1) * P, :])
        for a in range(KT):
            tr = psum_tr.tile([P, P], F32R, tag="tr", name=f"tr_{tag}_{i}_{a}")
            nc.tensor.transpose(tr[:], xin[:, a * P : (a + 1) * P], ident[:])
            nc.any.tensor_copy(out=dst[:, a, i * P : (i + 1) * P], in_=tr[:])


def build_nc(n_iters=1, mv=MV_DEFAULT, mq=None):
    """mv: key/seq extent (multiple of 128); mq: query extent (multiple of
    32, <= mv) — query rows past mq are pad and never computed."""
    if mq is None:
        mq = MQ_DEFAULT if mv == MV_DEFAULT else mv
    mt = mv // P
    qch = _qchunks(mq)
    kch = _qchunks(mv)

    nc = bacc.Bacc(trn_type="TRN2", target_bir_lowering=False, debug=False)

    x_d = nc.dram_tensor("x", [mv, C], F32R, kind="ExternalInput").ap()
    t_d = nc.dram_tensor("t", [mv, C], F32R, kind="ExternalInput").ap()
    mask_d = nc.dram_tensor("mask", [1, mv], F32, kind="ExternalInput").ap()
    g1_d = nc.dram_tensor("G1", [C, C], F32R, kind="ExternalInput").ap()
    g1b_d = nc.dram_tensor("g1b", [1, C], F32, kind="ExternalInput").ap()
    wv_d = nc.dram_tensor("Wv", [C, C], F32R, kind="ExternalInput").ap()
    bv_d = nc.dram_tensor("bv", [1, C], F32, kind="ExternalInput").ap()
    g2b_d = nc.dram_tensor("g2b", [1, C], F32, kind="ExternalInput").ap()
    wcv_d = nc.dram_tensor("Wcv", [C, C], F32R, kind="ExternalInput").ap()
    bcv_d = nc.dram_tensor("bcv", [1, C], F32, kind="ExternalInput").ap()
    out_d = nc.dram_tensor("out", [mv, C], F32, kind="ExternalOutput").ap()
    rowsum_d = nc.dram_tensor("rowsum", [1, mv], F32, kind="ExternalOutput").ap()

    g1_t = g1_d.rearrange("(a p) n -> p a n", p=P)  # [P, KT, C]
    wv_t = wv_d.rearrange("(a p) n -> p a n", p=P)
    wcv_t = wcv_d.rearrange("(a p) n -> p a n", p=P)

    with tile.TileContext(nc) as tc, ExitStack() as ctx:
        const = ctx.enter_context(tc.tile_pool(name="const", bufs=1))
        acts = ctx.enter_context(tc.tile_pool(name="acts", bufs=1))
        wpool = ctx.enter_context(tc.tile_pool(name="wpool", bufs=1))
        io = ctx.enter_context(tc.tile_pool(name="io", bufs=1))
        psum_main = ctx.enter_context(tc.tile_pool(name="psum_main", bufs=2, space="PSUM"))

        # ---- constants ----
        ident32 = const.tile([P, P], F32, tag="ident32", name="ident32")
        make_identity(nc, ident32[:])
        ident = const.tile([P, P], F32R, tag="ident", name="ident")
        nc.vector.tensor_copy(out=ident[:], in_=ident32[:])

        mask_t = const.tile([P, mt], F32, tag="mask_t", name="mask_t")
        nc.sync.dma_start(mask_t[:], mask_d[0].rearrange("(a p) -> p a", p=P))
        colb = const.tile([P, mt], F32, tag="colb", name="colb")
        nc.vector.tensor_scalar(
            colb[:], mask_t[:], 10000.0 / SCALE, -10000.0 / SCALE, AL.mult, AL.add
        )

        rm_scaled = const.tile([P, mv], F32, tag="rm_scaled", name="rm_scaled")
        nc.sync.dma_start(rm_scaled[:], mask_d.partition_broadcast(P))
        nc.vector.tensor_scalar_mul(rm_scaled[:], rm_scaled[:], SCALE)

        ones32 = const.tile([P, 1], F32, tag="ones32", name="ones32")
        nc.gpsimd.memset(ones32[:], 1.0)
        ones_r = const.tile([P, 1], F32R, tag="ones_r", name="ones_r")
        nc.vector.tensor_copy(out=ones_r[:], in_=ones32[:])
        ones_row32 = const.tile([1, P], F32, tag="ones_row32", name="ones_row32")
        nc.gpsimd.memset(ones_row32[:], 1.0)
        ones_row_r = const.tile([1, P], F32R, tag="ones_row_r", name="ones_row_r")
        nc.vector.tensor_copy(out=ones_row_r[:], in_=ones_row32[:])

        # per-partition bias columns (d on partitions)
        g1b_col = const.tile([P, KT], F32, tag="g1b_col", name="g1b_col")
        nc.sync.dma_start(g1b_col[:], g1b_d[0, :].rearrange("(a p) -> p a", p=P))
        g2b_col = const.tile([P, KT], F32, tag="g2b_col", name="g2b_col")
        nc.sync.dma_start(g2b_col[:], g2b_d[0, :].rearrange("(a p) -> p a", p=P))

        # ---- big activation tensors ----
        for _it in range(n_iters):
            _body_iter(nc, tc, ctx, acts, wpool, io, const, psum_main,
                       x_d, t_d, g1_t, wv_t, wcv_t,
                       bv_d, bcv_d, out_d, rowsum_d,
                       ident, colb, rm_scaled, ones_r, ones_row_r,
                       g1b_col, g2b_col, _it, mt, qch, kch, mv, mq)

    nc.compile()
    return nc


def _body_iter(nc, tc, ctx, acts, wpool, io, const, psum_main,
               x_d, t_d, g1_t, wv_t, wcv_t,
               bv_d, bcv_d, out_d, rowsum_d,
               ident, colb, rm_scaled, ones_r, ones_row_r,
               g1b_col, g2b_col, it, mt, qch, kch, mv, mq):
    if True:
        # scores are computed via the folded form s = (x@G + Wk@bq) . x
        # (softmax is invariant to the dropped per-query terms), so the
        # attention KEYS are the raw transposed inputs xT / tT and the k/ck
        # projections never happen.
        xT = acts.tile([P, KT, mv], F32R, tag="xT", name="xT")  # x^T
        qT = acts.tile([P, KT, mv], F32R, tag="qT", name="qT")  # g1^T
        vn = acts.tile([P, mt, C], F32R, tag="vn", name="vn")
        o1T = acts.tile([P, KT, mv], F32R, tag="oT", name="o1T")

        # ---- phase A: transpose x ----
        psum_tr = tc.alloc_tile_pool(name="psum_tr", bufs=6, space="PSUM")
        _transpose_in(nc, io, psum_tr, x_d, xT, ident, f"x{it}", mt)

        # ---- phase B: g1 + v projections ----
        bias_bc = wpool.tile([P, C], F32, tag="bbc", name="vbias_bc")
        nc.sync.dma_start(bias_bc[:], bv_d[0:1, :].partition_broadcast(P))

        for d in range(KT):
            w = wpool.tile([P, KT, P], F32R, tag="ws", name=f"wsg1_{d}", bufs=3)
            nc.sync.dma_start(w[:], g1_t[:, :, d * P : (d + 1) * P])
            for (off, fw) in qch:
                ps = psum_main.tile([P, 512], F32, tag="st", name=f"ps_g1_{d}_{off}")
                for a in range(KT):
                    nc.tensor.matmul(
                        ps[:, :fw],
                        w[:, a, :],
                        xT[:, a, off : off + fw],
                        start=(a == 0),
                        stop=(a == KT - 1),
                    )
                nc.any.tensor_scalar_add(
                    qT[:, d, off : off + fw], ps[:, :fw], g1b_col[:, d : d + 1]
                )

        vw = wpool.tile([P, KT, C], F32R, tag="vw", name="vw_v")
        nc.sync.dma_start(vw[:], wv_t[:])
        _proj_natural(nc, xT, vw, vn, bias_bc, psum_main, mt)

        # ---- phase A2: transpose t (kT slot; live through attn2) ----
        tT = acts.tile([P, KT, mv], F32R, tag="kT", name="tT")
        _transpose_in(nc, io, psum_tr, t_d, tT, ident, f"t{it}", mt)
        psum_tr.release()

        psum_att = tc.alloc_tile_pool(name="psum_att", bufs=6, space="PSUM")

        # ---- phase C/D: attention 1 (keys = xT) ----
        class _AttPsum:
            def tile(self, shape, dtype, tag, name):
                pool = psum_att if tag == "po" else psum_main
                return pool.tile(shape, dtype, tag=tag, name=name)

        att_psum = _AttPsum()
        rbcs1 = _attention(
            nc, io, att_psum, qT, xT, vn, o1T, colb, rm_scaled,
            ones_r, ones_row_r, "a1", mt, qch,
        )

        # ---- phase E: normalize raw g2 (attn1 accumulated p1 @ (v@G2)
        # directly, so g2 = o1T_raw * recip + g2b is a pure vector pass) ----
        g2T = acts.tile([P, KT, mv], F32R, tag="qT", name="g2T")
        for d in range(KT):
            for ci, (off, fw) in enumerate(qch):
                dst = g2T[:, d, off : off + fw]
                nc.any.tensor_mul(
                    out=dst, in0=o1T[:, d, off : off + fw], in1=rbcs1[ci]
                )
                nc.any.tensor_scalar_add(dst, dst, g2b_col[:, d : d + 1])

        # ---- phase F: cv' projection from t (into vn slot) ----
        cvn = acts.tile([P, mt, C], F32R, tag="vn", name="cvn")
        cvw = wpool.tile([P, KT, C], F32R, tag="vw", name="vw_cv")
        nc.sync.dma_start(cvw[:], wcv_t[:])
        cv_bias_bc = wpool.tile([P, C], F32, tag="bbc", name="cvbias_bc")
        nc.sync.dma_start(cv_bias_bc[:], bcv_d[0:1, :].partition_broadcast(P))
        _proj_natural(nc, tT, cvw, cvn, cv_bias_bc, psum_main, mt)

        # ---- phase G: attention 2 (keys = tT), natural out straight to
        # DRAM; host normalizes rows and adds the ffn bias ----
        psum_att.release()
        psum_att2 = tc.alloc_tile_pool(name="psum_att2", bufs=2, space="PSUM")
        _attention_nat(
            nc, io, psum_att2, psum_main, g2T, tT, cvn, out_d, rowsum_d,
            colb, rm_scaled, ones_r, "a2", mt, qch[::-1],
        )
        psum_att2.release()


_NC_CACHE = {}


def _get_nc(dims=(MV_DEFAULT, MQ_DEFAULT)):
    if dims not in _NC_CACHE:
        _NC_CACHE[dims] = build_nc(mv=dims[0], mq=dims[1])
    return _NC_CACHE[dims]


def prep_inputs(layout_x, text_x, mask, Wqkv, bqkv, Wq, bq, Wkv, bkv, Wffn, bffn):
    """Host-side valid-row compaction.

    Returns (mv, in_maps, idxs, fill) where in_maps feeds the device kernel
    (compacted to mv rows per core), idxs[b] are the valid row indices, and
    fill[b] is the output vector for every masked row of batch b.
    """
    layout_x = np.ascontiguousarray(np.asarray(layout_x, dtype=np.float32))
    text_x = np.ascontiguousarray(np.asarray(text_x, dtype=np.float32))
    mask = np.ascontiguousarray(np.asarray(mask, dtype=np.float32))
    Wqkv = np.ascontiguousarray(np.asarray(Wqkv, dtype=np.float32))
    bqkv = np.ascontiguousarray(np.asarray(bqkv, dtype=np.float32)).reshape(1, 3 * C)
    Wq = np.ascontiguousarray(np.asarray(Wq, dtype=np.float32))
    bq = np.ascontiguousarray(np.asarray(bq, dtype=np.float32)).reshape(1, C)
    Wkv = np.ascontiguousarray(np.asarray(Wkv, dtype=np.float32))
    bkv = np.ascontiguousarray(np.asarray(bkv, dtype=np.float32)).reshape(1, 2 * C)
    Wffn = np.ascontiguousarray(np.asarray(Wffn, dtype=np.float32))
    bffn = np.ascontiguousarray(np.asarray(bffn, dtype=np.float32)).reshape(1, C)

    B = layout_x.shape[0]
    assert B == N_CORES

    idxs = [np.nonzero(mask[b] != 0)[0] for b in range(B)]
    nvs = [len(ix) for ix in idxs]
    mv = min(M, max(P, -(-max(nvs) // P) * P))
    mq = min(mv, max(P, -(-max(nvs) // 32) * 32))

    # masked-row output: uniform attn2 over ALL cv rows, then ffn
    tx_mean = text_x.astype(np.float64).mean(axis=1)  # [B, C]
    cv_mean = tx_mean @ Wkv[:, C:].astype(np.float64) + bkv[0, C:].astype(np.float64)
    fill = (cv_mean @ Wffn.astype(np.float64) + bffn[0].astype(np.float64)).astype(
        np.float32
    )  # [B, C]

    # fold q/k projections: s = (x@Wq + bq).(x@Wk + bk) == (x@G + Wk@bq).x
    # up to per-query additive constants that cancel in softmax.  Further
    # folds (all exact): attn1's v carries G2 (so attn1's output IS raw g2),
    # attn2's cv carries Wffn (so attn2's output IS the raw final numerator;
    # the host divides by rowsum and adds bffn).
    G1 = np.ascontiguousarray(Wqkv[:, :C] @ Wqkv[:, C : 2 * C].T)
    g1b = (Wqkv[:, C : 2 * C] @ bqkv[0, :C]).reshape(1, C)
    G2 = Wq @ Wkv[:, :C].T
    g2b = (Wkv[:, :C] @ bq[0]).reshape(1, C)
    Wv2 = np.ascontiguousarray(Wqkv[:, 2 * C :] @ G2)
    bv2 = (bqkv[0, 2 * C :] @ G2).reshape(1, C)
    Wcv2 = np.ascontiguousarray(Wkv[:, C:] @ Wffn)
    bcv2 = (bkv[0, C:] @ Wffn).reshape(1, C)

    in_maps = []
    for b in range(B):
        nv = nvs[b]
        x_c = np.zeros((mv, C), np.float32)
        x_c[:nv] = layout_x[b][idxs[b]]
        t_c = np.zeros((mv, C), np.float32)
        t_c[:nv] = text_x[b][idxs[b]]
        m_c = np.zeros((1, mv), np.float32)
        m_c[0, :nv] = 1.0
        in_maps.append(
            {
                "x": x_c,
                "t": t_c,
                "mask": m_c,
                "G1": G1,
                "g1b": g1b,
                "Wv": Wv2,
                "bv": bv2,
                "g2b": g2b,
                "Wcv": Wcv2,
                "bcv": bcv2,
            }
        )
    return (mv, mq), in_maps, idxs, fill, bffn[0]


def finish_output(dev_outs, dev_rowsums, idxs, fill, bffn):
    """Normalize + scatter compacted device outputs to [B, M, C]."""
    B = len(idxs)
    out = np.empty((B, M, C), np.float32)
    for b in range(B):
        nv = len(idxs[b])
        out[b, :, :] = fill[b]
        raw = dev_outs[b][:nv].astype(np.float64)
        rs = dev_rowsums[b].reshape(-1)[:nv].astype(np.float64)
        out[b, idxs[b], :] = raw / rs[:, None] + bffn.astype(np.float64)
    return out


def kernel(
    layout_x, text_x, mask, Wqkv, bqkv, Wq, bq, Wkv, bkv, Wffn, bffn
):
    dims, in_maps, idxs, fill, bffn_v = prep_inputs(
        layout_x, text_x, mask, Wqkv, bqkv, Wq, bq, Wkv, bkv, Wffn, bffn
    )
    nc = _get_nc(dims)
    res = run_bass_kernel_spmd(nc, in_maps, core_ids=list(range(N_CORES)))
    return finish_output(
        [res.results[b]["out"] for b in range(N_CORES)],
        [res.results[b]["rowsum"] for b in range(N_CORES)],
        idxs, fill, bffn_v,
    )


# revision 43
# speedup vs baseline: 2.8868x; 1.0232x over previous
"""Trainium2 Bass kernel for nn_Attention_kv (dense transformer block).

Sharding: data-parallel over batch B=8 across the 8 NeuronCores — one batch
element per core, no collectives (host scatters inputs / stacks outputs).

Valid-row compaction (host side): ~50% of sequence positions have mask==0.
Masked QUERY rows of both attentions get uniform attention over ALL keys, so
every masked row of the final output equals one per-batch vector
  fill[b] = (mean_M(text_x[b]) @ Wkv[:,C:] + bkv[C:]) @ Wffn + bffn
computable on the host in microseconds.  Valid rows never depend on masked
rows: an invalid key's attention weight is exp(s*scale - 10000) == 0.0 in
fp32 (hard underflow), identical to the reference's jnp.where(-10000) +
softmax.  So the device only sees the gathered valid rows, padded to a
multiple of 128 (634 max over batches -> MV=640 for the graded inputs), with
a recomputed 1/0 mask for the pad tail; the host scatters device rows back
into the valid positions and broadcast-fills masked rows with fill[b].
This removes ~48% of all PE work (projections scale by MV/M, attention by
(MV/M)^2) and is exact up to fp32 reassociation.

Projection folding (host side, all exact): softmax is invariant to
per-query additive score constants, so
  scores1 = (x@Wq1 + bq1).(x@Wk1 + bk1)  ==  (x@G1 + Wk1@bq1) . x
with G1 = Wq1@Wk1^T — the k-projection vanishes and attention keys are the
raw transposed input.  Likewise for attention 2 (G2 = Wq@Wkc^T, keys = t).
Further, per-row scalars commute with matmuls, so
  g2 = ((p1@v)*recip)@G2 + b  ==  (p1@(v@G2))*recip + b
means attn1 accumulates with the folded v' = x@(Wv@G2) and its output IS
raw g2 (the g2 projection becomes a vector normalize pass), and since
normalized attention rows sum to 1,
  out = (p2@cv)@Wffn + bffn  ==  p2@(cv@Wffn) + bffn
means attn2 accumulates with cv' = t@(Wcv@Wffn) and its raw output IS the
final numerator — the ffn phase vanishes; the host divides by the exported
row sums and adds bffn.

Per-core pipeline (seq MV, dim C=768):
  x^T via PE 128x128 transposes -> g1 projection (transposed [d, seq]) and
  v' projection (natural) -> t^T transposes
  -> attn1: scores TRANSPOSED S^T[sk, sq] with keys = x^T; max-free masked
     softmax (additive -10000 key mask + multiplicative query-mask zeroing);
     row sums via PE ones-matmul; raw-g2^T accumulated across 6 PSUM banks
     flash-style; normalization deferred into a vector pass
  -> cv' projection (from t) -> attn2 with keys = t^T, output accumulated
     NATURALLY per 128-query sub-tile and DMA'd raw; row sums exported.

All matmuls run in float32r (TF32-like PE datapath, 1 cycle/row — measured
numerically identical to the fp32 4-cycle/row path on this hardware).
"""

import sys

sys.path.insert(0, "/opt/trn_rl_repo")

from contextlib import ExitStack

import numpy as np

import concourse.bass as bass
import concourse.mybir as mybir
import concourse.tile as tile
from concourse import bacc
from concourse.bass_utils import run_bass_kernel_spmd
from concourse.masks import make_identity

P = 128
M = 1024  # full sequence length per batch element (host side)
C = 768  # model dim
KT = C // P  # 6 contraction tiles
MV_DEFAULT = 640  # compacted device seq len for the graded inputs
MQ_DEFAULT = 544  # compacted query extent (max 534 valid rows per batch)
SCALE = float(C) ** -0.5
NEG = -10000.0

F32 = mybir.dt.float32
F32R = mybir.dt.float32r
AL = mybir.AluOpType
AF = mybir.ActivationFunctionType

N_CORES = 8


def _qchunks(mv):
    """Split the device seq dim into free-dim chunks of <= 512 (one PSUM
    bank of fp32)."""
    out = []
    off = 0
    while off < mv:
        w = min(512, mv - off)
        out.append((off, w))
        off += w
    return out


def _proj_natural(nc, lhs_src, w_rhs, dst, bias_bc, psum_pool, mt):
    """dst[:, i, :] (shape [P, mt, C]) = src @ W + bias.

    lhs_src: AP [P, KT, MV] (x^T layout, f32r) -- lhsT tiles [P, 128]
    w_rhs: AP [P, KT, C] (weight, f32r) -- rhs tiles [P, chunk]
    bias_bc: AP [P, C] broadcast bias or None
    """
    chunks = [(0, 512), (512, 256)]
    for i in range(mt):
        pss = []
        for (off, w) in chunks:
            ps = psum_pool.tile([P, 512], F32, tag="st", name=f"ps_v_{i}_{off}")
            for a in range(KT):
                nc.tensor.matmul(
                    ps[:, :w],
                    lhs_src[:, a, i * P : (i + 1) * P],
                    w_rhs[:, a, off : off + w],
                    start=(a == 0),
                    stop=(a == KT - 1),
                )
            pss.append(ps)
        for (off, w), ps in zip(chunks, pss):
            if bias_bc is not None:
                nc.any.tensor_add(
                    out=dst[:, i, off : off + w],
                    in0=ps[:, :w],
                    in1=bias_bc[:, off : off + w],
                )
            else:
                nc.any.tensor_copy(out=dst[:, i, off : off + w], in_=ps[:, :w])


def _attention(nc, io, psum_pool, qT, kT, vn, outT, colb, rm_scaled,
               ones_r, ones_row_r, label, mt, qch,
               recip_col=None, dram_pool=None):
    # qch are the QUERY chunks and may stop short of the key extent
    # (mt*P): pad queries past max-valid-rows are never consumed.
    """outT[:, d, :] = (UNNORMALIZED attn numerator)^T, [P, KT, MV] f32r.

    Normalization is deferred to the consumer: returns per-chunk rbc
    broadcast tile slices [P, fw] (1/rowsum along free sq) unless recip_col
    is given, in which case recip values are instead written into
    recip_col ([P, mt] column layout) and no bcast is made.

    qT, kT: [P, KT, MV] f32r (d on partitions); vn: [P, mt, C] f32r.
    colb: [P, mt] fp32 = (mask-1)*10000/scale along sk partitions.
    rm_scaled: [P, MV] fp32 = mask*scale broadcast (varies along free sq).
    """
    rbcs = []
    for c, (off, fw) in enumerate(qch):
        sq = slice(off, off + fw)
        # out^T accumulators: 6 banks
        pos = [
            psum_pool.tile([P, 512], F32, tag="po", name=f"po_{label}_{c}_{d}")
            for d in range(KT)
        ]
        p_tiles = []
        prev = None  # (j, p_j) pending out^T matmuls
        for j in range(mt):
            st = psum_pool.tile([P, 512], F32, tag="st", name=f"st_{label}_{c}_{j}")
            for a in range(KT):
                nc.tensor.matmul(
                    st[:, :fw],
                    kT[:, a, j * P : (j + 1) * P],
                    qT[:, a, sq],
                    start=(a == 0),
                    stop=(a == KT - 1),
                )
            # masked = (S^T + colb_j) * rm_scaled ; exp
            mk = io.tile([P, 512], F32, tag="mk", name=f"mk_{label}_{c}_{j}", bufs=2)
            nc.vector.scalar_tensor_tensor(
                out=mk[:, :fw],
                in0=st[:, :fw],
                scalar=colb[:, j : j + 1],
                in1=rm_scaled[:, sq],
                op0=AL.add,
                op1=AL.mult,
            )
            pj = io.tile([P, 512], F32R, tag="pp", name=f"p_{label}_{c}_{j}", bufs=mt + 2)
            nc.scalar.activation(pj[:, :fw], mk[:, :fw], AF.Exp)
            p_tiles.append(pj)
            if prev is not None:
                jj, pprev = prev
                for d in range(KT):
                    nc.tensor.matmul(
                        pos[d][:, :fw],
                        vn[:, jj, d * P : (d + 1) * P],
                        pprev[:, :fw],
                        start=(jj == 0),
                        stop=False,
                    )
            prev = (j, pj)
        jj, pprev = prev
        for d in range(KT):
            nc.tensor.matmul(
                pos[d][:, :fw],
                vn[:, jj, d * P : (d + 1) * P],
                pprev[:, :fw],
                start=(jj == 0),
                stop=True,
            )
        # row sums over sk (partitions + tiles) via ones-matmul
        rs = psum_pool.tile([P, 512], F32, tag="st", name=f"rs_{label}_{c}")
        for j in range(mt):
            nc.tensor.matmul(
                rs[0:1, :fw],
                ones_r[:],
                p_tiles[j][:, :fw],
                start=(j == 0),
                stop=(j == mt - 1),
            )
        recip = io.tile([1, 512], F32R, tag="recip", name=f"recip_{label}_{c}", bufs=2)
        with nc.allow_low_precision(reason="f32r recip feeds f32r bcast matmul"):
            nc.vector.reciprocal(recip[:, :fw], rs[0:1, :fw])
        if recip_col is None:
            # broadcast recip across partitions via K=1 f32r matmul
            bc = psum_pool.tile([P, 512], F32, tag="st", name=f"bc_{label}_{c}")
            nc.tensor.matmul(bc[:, :fw], ones_row_r[:], recip[:, :fw], start=True, stop=True)
            rbc = io.tile([P, 512], F32, tag="rbc", name=f"rbc_{label}_{c}", bufs=4)
            nc.vector.tensor_copy(out=rbc[:, :fw], in_=bc[:, :fw])
            rbcs.append(rbc[:, :fw])
        else:
            # column layout recip_col[p, a] = 1/rowsum[sq = off + a*P + p]
            # via a DRAM bounce (free->partition reshuffles need DMA via DRAM)
            scr = dram_pool.tile([1, 512], F32, tag="rscr", name=f"rscr_{label}_{c}", bufs=2)
            nc.sync.dma_start(scr[:, :fw], recip[:, :fw].bitcast(F32))
            nfull = fw // P
            rem = fw - nfull * P
            if nfull:
                nc.sync.dma_start(
                    recip_col[:, off // P : off // P + nfull],
                    scr[0, : nfull * P].rearrange("(a p) -> p a", p=P),
                )
            if rem:
                nc.sync.dma_start(
                    recip_col[0:rem, off // P + nfull : off // P + nfull + 1],
                    scr[0, nfull * P : fw].rearrange("(a p) -> p a", p=rem),
                )
        # UNNORMALIZED copyback (releases psum_o banks immediately)
        for d in range(KT):
            nc.vector.tensor_copy(out=outT[:, d, sq], in_=pos[d][:, :fw])
    return rbcs


def _attention_nat(nc, io, psum_att, psum_main, qT, kT, vn, out_d,
                   colb, rm_scaled, label, mt, qch):
    """Natural-output attention: out_d[sq, 0:C] = UNNORMALIZED numerator
    p @ vn (rows on partitions), out_d[sq, C] = per-query exp-sum (vn
    carries a ones column at C, so the row sums ride along in the same
    accumulation).  Normalization and the final bias happen on the host.

    vn here is the folded v' = t @ (Wcv @ Wffn) + bcv @ Wffn, so this fuses
    attention-2's output accumulation with the reference's trailing ffn.
    """
    for (off, fw) in qch:
        sq = slice(off, off + fw)
        subs = []
        q0 = 0
        while q0 < fw:
            subs.append((q0, min(P, fw - q0)))
            q0 += P
        # all score/exp tiles of the chunk first (they all feed rowsum)
        p_tiles = []
        for j in range(mt):
            st = psum_main.tile([P, 512], F32, tag="st", name=f"st_{label}_{off}_{j}")
            for a in range(KT):
                nc.tensor.matmul(
                    st[:, :fw],
                    kT[:, a, j * P : (j + 1) * P],
                    qT[:, a, sq],
                    start=(a == 0),
                    stop=(a == KT - 1),
                )
            mk = io.tile([P, 512], F32, tag="mk", name=f"mk_{label}_{off}_{j}", bufs=2)
            nc.vector.scalar_tensor_tensor(
                out=mk[:, :fw],
                in0=st[:, :fw],
                scalar=colb[:, j : j + 1],
                in1=rm_scaled[:, sq],
                op0=AL.add,
                op1=AL.mult,
            )
            pj = io.tile([P, 512], F32R, tag="pp", name=f"p_{label}_{off}_{j}", bufs=mt + 2)
            nc.scalar.activation(pj[:, :fw], mk[:, :fw], AF.Exp)
            p_tiles.append(pj)
        # output accumulation per 128-query sub-tile (p^T slices as lhsT),
        # [P,512]+[P,257] psum pairs to stay within PSUM banks; column C of
        # vn is ones, so output column C accumulates the row sums
        for si, (qo, qw) in enumerate(subs):
            pa = psum_att.tile([P, 512], F32, tag="pona", name=f"pona_{label}_{off}_{si}")
            pb = psum_att.tile([P, 264], F32, tag="ponb", name=f"ponb_{label}_{off}_{si}")
            for jj in range(mt):
                nc.tensor.matmul(
                    pa[0:qw, :],
                    p_tiles[jj][:, qo : qo + qw],
                    vn[:, jj, 0:512],
                    start=(jj == 0),
                    stop=(jj == mt - 1),
                )
                # f32r matmuls need even free sizes: accumulate 258 wide
                # (col C is ones -> rowsum, col C+1 is zeros -> discarded)
                nc.tensor.matmul(
                    pb[0:qw, : C + 2 - 512],
                    p_tiles[jj][:, qo : qo + qw],
                    vn[:, jj, 512 : C + 2],
                    start=(jj == 0),
                    stop=(jj == mt - 1),
                )
            fin = io.tile([P, C + 8], F32, tag="fin", name=f"fin_{label}_{off}_{si}", bufs=2)
            nc.vector.tensor_copy(out=fin[0:qw, 0:512], in_=pa[0:qw, :])
            nc.vector.tensor_copy(
                out=fin[0:qw, 512 : C + 1], in_=pb[0:qw, : C + 1 - 512]
            )
            nc.sync.dma_start(
                out_d[off + qo : off + qo + qw, :], fin[0:qw, : C + 1]
            )


def _transpose_in(nc, io, psum_tr, src_dram, dst, ident, tag, mt):
    """dst [P, KT, MV] (f32r) = src^T, via PE 128x128 transposes."""
    for i in range(mt):
        xin = io.tile([P, C], F32R, tag="xin", name=f"xin_{tag}_{i}", bufs=3)
        nc.sync.dma_start(xin[:], src_dram[i * P : (i + 1) * P, :])
        for a in range(KT):
            tr = psum_tr.tile([P, P], F32R, tag="tr", name=f"tr_{tag}_{i}_{a}")
            nc.tensor.transpose(tr[:], xin[:, a * P : (a + 1) * P], ident[:])
            nc.any.tensor_copy(out=dst[:, a, i * P : (i + 1) * P], in_=tr[:])


def build_nc(n_iters=1, mv=MV_DEFAULT, mq=None):
    """mv: key/seq extent (multiple of 128); mq: query extent (multiple of
    32, <= mv) — query rows past mq are pad and never computed."""
    if mq is None:
        mq = MQ_DEFAULT if mv == MV_DEFAULT else mv
    mt = mv // P
    qch = _qchunks(mq)
    kch = _qchunks(mv)

    nc = bacc.Bacc(trn_type="TRN2", target_bir_lowering=False, debug=False)

    x_d = nc.dram_tensor("x", [mv, C], F32R, kind="ExternalInput").ap()
    t_d = nc.dram_tensor("t", [mv, C], F32R, kind="ExternalInput").ap()
    mask_d = nc.dram_tensor("mask", [1, mv], F32, kind="ExternalInput").ap()
    g1_d = nc.dram_tensor("G1", [C, C], F32R, kind="ExternalInput").ap()
    g1b_d = nc.dram_tensor("g1b", [1, C], F32, kind="ExternalInput").ap()
    wv_d = nc.dram_tensor("Wv", [C, C], F32R, kind="ExternalInput").ap()
    bv_d = nc.dram_tensor("bv", [1, C], F32, kind="ExternalInput").ap()
    g2b_d = nc.dram_tensor("g2b", [1, C], F32, kind="ExternalInput").ap()
    wcv_d = nc.dram_tensor("Wcv", [C, C], F32R, kind="ExternalInput").ap()
    bcv_d = nc.dram_tensor("bcv", [1, C], F32, kind="ExternalInput").ap()
    out_d = nc.dram_tensor("out", [mv, C + 1], F32, kind="ExternalOutput").ap()

    g1_t = g1_d.rearrange("(a p) n -> p a n", p=P)  # [P, KT, C]
    wv_t = wv_d.rearrange("(a p) n -> p a n", p=P)
    wcv_t = wcv_d.rearrange("(a p) n -> p a n", p=P)

    with tile.TileContext(nc) as tc, ExitStack() as ctx:
        const = ctx.enter_context(tc.tile_pool(name="const", bufs=1))
        acts = ctx.enter_context(tc.tile_pool(name="acts", bufs=1))
        wpool = ctx.enter_context(tc.tile_pool(name="wpool", bufs=1))
        io = ctx.enter_context(tc.tile_pool(name="io", bufs=1))
        psum_main = ctx.enter_context(tc.tile_pool(name="psum_main", bufs=2, space="PSUM"))

        # ---- constants ----
        ident32 = const.tile([P, P], F32, tag="ident32", name="ident32")
        make_identity(nc, ident32[:])
        ident = const.tile([P, P], F32R, tag="ident", name="ident")
        nc.vector.tensor_copy(out=ident[:], in_=ident32[:])

        mask_t = const.tile([P, mt], F32, tag="mask_t", name="mask_t")
        nc.sync.dma_start(mask_t[:], mask_d[0].rearrange("(a p) -> p a", p=P))
        colb = const.tile([P, mt], F32, tag="colb", name="colb")
        nc.vector.tensor_scalar(
            colb[:], mask_t[:], 10000.0 / SCALE, -10000.0 / SCALE, AL.mult, AL.add
        )

        rm_scaled = const.tile([P, mv], F32, tag="rm_scaled", name="rm_scaled")
        nc.sync.dma_start(rm_scaled[:], mask_d.partition_broadcast(P))
        nc.vector.tensor_scalar_mul(rm_scaled[:], rm_scaled[:], SCALE)

        ones32 = const.tile([P, 1], F32, tag="ones32", name="ones32")
        nc.gpsimd.memset(ones32[:], 1.0)
        ones_r = const.tile([P, 1], F32R, tag="ones_r", name="ones_r")
        nc.vector.tensor_copy(out=ones_r[:], in_=ones32[:])
        ones_row32 = const.tile([1, P], F32, tag="ones_row32", name="ones_row32")
        nc.gpsimd.memset(ones_row32[:], 1.0)
        ones_row_r = const.tile([1, P], F32R, tag="ones_row_r", name="ones_row_r")
        nc.vector.tensor_copy(out=ones_row_r[:], in_=ones_row32[:])

        # per-partition bias columns (d on partitions)
        g1b_col = const.tile([P, KT], F32, tag="g1b_col", name="g1b_col")
        nc.sync.dma_start(g1b_col[:], g1b_d[0, :].rearrange("(a p) -> p a", p=P))
        g2b_col = const.tile([P, KT], F32, tag="g2b_col", name="g2b_col")
        nc.sync.dma_start(g2b_col[:], g2b_d[0, :].rearrange("(a p) -> p a", p=P))

        # ---- big activation tensors ----
        for _it in range(n_iters):
            _body_iter(nc, tc, ctx, acts, wpool, io, const, psum_main,
                       x_d, t_d, g1_t, wv_t, wcv_t,
                       bv_d, bcv_d, out_d,
                       ident, colb, rm_scaled, ones_r, ones_row_r,
                       g1b_col, g2b_col, _it, mt, qch, kch, mv, mq)

    nc.compile()
    return nc


def _body_iter(nc, tc, ctx, acts, wpool, io, const, psum_main,
               x_d, t_d, g1_t, wv_t, wcv_t,
               bv_d, bcv_d, out_d,
               ident, colb, rm_scaled, ones_r, ones_row_r,
               g1b_col, g2b_col, it, mt, qch, kch, mv, mq):
    if True:
        # scores are computed via the folded form s = (x@G + Wk@bq) . x
        # (softmax is invariant to the dropped per-query terms), so the
        # attention KEYS are the raw transposed inputs xT / tT and the k/ck
        # projections never happen.
        xT = acts.tile([P, KT, mv], F32R, tag="xT", name="xT")  # x^T
        qT = acts.tile([P, KT, mv], F32R, tag="qT", name="qT")  # g1^T
        vn = acts.tile([P, mt, C], F32R, tag="vn", name="vn")
        o1T = acts.tile([P, KT, mv], F32R, tag="oT", name="o1T")

        # ---- phase A: transpose x ----
        psum_tr = tc.alloc_tile_pool(name="psum_tr", bufs=6, space="PSUM")
        _transpose_in(nc, io, psum_tr, x_d, xT, ident, f"x{it}", mt)

        # ---- phase B: g1 + v projections ----
        bias_bc = wpool.tile([P, C], F32, tag="bbc", name="vbias_bc")
        nc.sync.dma_start(bias_bc[:], bv_d[0:1, :].partition_broadcast(P))

        for d in range(KT):
            w = wpool.tile([P, KT, P], F32R, tag="ws", name=f"wsg1_{d}", bufs=3)
            nc.sync.dma_start(w[:], g1_t[:, :, d * P : (d + 1) * P])
            for (off, fw) in qch:
                ps = psum_main.tile([P, 512], F32, tag="st", name=f"ps_g1_{d}_{off}")
                for a in range(KT):
                    nc.tensor.matmul(
                        ps[:, :fw],
                        w[:, a, :],
                        xT[:, a, off : off + fw],
                        start=(a == 0),
                        stop=(a == KT - 1),
                    )
                nc.any.tensor_scalar_add(
                    qT[:, d, off : off + fw], ps[:, :fw], g1b_col[:, d : d + 1]
                )

        vw = wpool.tile([P, KT, C], F32R, tag="vw", name="vw_v")
        nc.sync.dma_start(vw[:], wv_t[:])
        _proj_natural(nc, xT, vw, vn, bias_bc, psum_main, mt)

        # ---- phase A2: transpose t (kT slot; live through attn2) ----
        tT = acts.tile([P, KT, mv], F32R, tag="kT", name="tT")
        _transpose_in(nc, io, psum_tr, t_d, tT, ident, f"t{it}", mt)
        psum_tr.release()

        psum_att = tc.alloc_tile_pool(name="psum_att", bufs=6, space="PSUM")

        # ---- phase C/D: attention 1 (keys = xT) ----
        class _AttPsum:
            def tile(self, shape, dtype, tag, name):
                pool = psum_att if tag == "po" else psum_main
                return pool.tile(shape, dtype, tag=tag, name=name)

        att_psum = _AttPsum()
        rbcs1 = _attention(
            nc, io, att_psum, qT, xT, vn, o1T, colb, rm_scaled,
            ones_r, ones_row_r, "a1", mt, qch,
        )

        # ---- phase E: normalize raw g2 (attn1 accumulated p1 @ (v@G2)
        # directly, so g2 = o1T_raw * recip + g2b is a pure vector pass) ----
        g2T = acts.tile([P, KT, mv], F32R, tag="qT", name="g2T")
        for d in range(KT):
            for ci, (off, fw) in enumerate(qch):
                dst = g2T[:, d, off : off + fw]
                nc.any.tensor_mul(
                    out=dst, in0=o1T[:, d, off : off + fw], in1=rbcs1[ci]
                )
                nc.any.tensor_scalar_add(dst, dst, g2b_col[:, d : d + 1])

        # ---- phase F: cv' projection from t (into vn slot); column C is
        # set to ones so attn2's accumulation also produces row sums ----
        cvn = acts.tile([P, mt, C + 8], F32R, tag="vn", name="cvn")
        cvw = wpool.tile([P, KT, C], F32R, tag="vw", name="vw_cv")
        nc.sync.dma_start(cvw[:], wcv_t[:])
        cv_bias_bc = wpool.tile([P, C], F32, tag="bbc", name="cvbias_bc")
        nc.sync.dma_start(cv_bias_bc[:], bcv_d[0:1, :].partition_broadcast(P))
        _proj_natural(nc, tT, cvw, cvn, cv_bias_bc, psum_main, mt)
        for i in range(mt):
            nc.gpsimd.memset(cvn[:, i, C : C + 1].bitcast(F32), 1.0)
            nc.gpsimd.memset(cvn[:, i, C + 1 : C + 2].bitcast(F32), 0.0)

        # ---- phase G: attention 2 (keys = tT), natural out straight to
        # DRAM; host normalizes rows and adds the ffn bias ----
        psum_att.release()
        psum_att2 = tc.alloc_tile_pool(name="psum_att2", bufs=2, space="PSUM")
        _attention_nat(
            nc, io, psum_att2, psum_main, g2T, tT, cvn, out_d,
            colb, rm_scaled, "a2", mt, qch[::-1],
        )
        psum_att2.release()


_NC_CACHE = {}


def _get_nc(dims=(MV_DEFAULT, MQ_DEFAULT)):
    if dims not in _NC_CACHE:
        _NC_CACHE[dims] = build_nc(mv=dims[0], mq=dims[1])
    return _NC_CACHE[dims]


def prep_inputs(layout_x, text_x, mask, Wqkv, bqkv, Wq, bq, Wkv, bkv, Wffn, bffn):
    """Host-side valid-row compaction.

    Returns (mv, in_maps, idxs, fill) where in_maps feeds the device kernel
    (compacted to mv rows per core), idxs[b] are the valid row indices, and
    fill[b] is the output vector for every masked row of batch b.
    """
    layout_x = np.ascontiguousarray(np.asarray(layout_x, dtype=np.float32))
    text_x = np.ascontiguousarray(np.asarray(text_x, dtype=np.float32))
    mask = np.ascontiguousarray(np.asarray(mask, dtype=np.float32))
    Wqkv = np.ascontiguousarray(np.asarray(Wqkv, dtype=np.float32))
    bqkv = np.ascontiguousarray(np.asarray(bqkv, dtype=np.float32)).reshape(1, 3 * C)
    Wq = np.ascontiguousarray(np.asarray(Wq, dtype=np.float32))
    bq = np.ascontiguousarray(np.asarray(bq, dtype=np.float32)).reshape(1, C)
    Wkv = np.ascontiguousarray(np.asarray(Wkv, dtype=np.float32))
    bkv = np.ascontiguousarray(np.asarray(bkv, dtype=np.float32)).reshape(1, 2 * C)
    Wffn = np.ascontiguousarray(np.asarray(Wffn, dtype=np.float32))
    bffn = np.ascontiguousarray(np.asarray(bffn, dtype=np.float32)).reshape(1, C)

    B = layout_x.shape[0]
    assert B == N_CORES

    idxs = [np.nonzero(mask[b] != 0)[0] for b in range(B)]
    nvs = [len(ix) for ix in idxs]
    mv = min(M, max(P, -(-max(nvs) // P) * P))
    mq = min(mv, max(P, -(-max(nvs) // 8) * 8))

    # masked-row output: uniform attn2 over ALL cv rows, then ffn
    tx_mean = text_x.astype(np.float64).mean(axis=1)  # [B, C]
    cv_mean = tx_mean @ Wkv[:, C:].astype(np.float64) + bkv[0, C:].astype(np.float64)
    fill = (cv_mean @ Wffn.astype(np.float64) + bffn[0].astype(np.float64)).astype(
        np.float32
    )  # [B, C]

    # fold q/k projections: s = (x@Wq + bq).(x@Wk + bk) == (x@G + Wk@bq).x
    # up to per-query additive constants that cancel in softmax.  Further
    # folds (all exact): attn1's v carries G2 (so attn1's output IS raw g2),
    # attn2's cv carries Wffn (so attn2's output IS the raw final numerator;
    # the host divides by rowsum and adds bffn).
    G1 = np.ascontiguousarray(Wqkv[:, :C] @ Wqkv[:, C : 2 * C].T)
    g1b = (Wqkv[:, C : 2 * C] @ bqkv[0, :C]).reshape(1, C)
    G2 = Wq @ Wkv[:, :C].T
    g2b = (Wkv[:, :C] @ bq[0]).reshape(1, C)
    Wv2 = np.ascontiguousarray(Wqkv[:, 2 * C :] @ G2)
    bv2 = (bqkv[0, 2 * C :] @ G2).reshape(1, C)
    Wcv2 = np.ascontiguousarray(Wkv[:, C:] @ Wffn)
    bcv2 = (bkv[0, C:] @ Wffn).reshape(1, C)

    in_maps = []
    for b in range(B):
        nv = nvs[b]
        x_c = np.zeros((mv, C), np.float32)
        x_c[:nv] = layout_x[b][idxs[b]]
        t_c = np.zeros((mv, C), np.float32)
        t_c[:nv] = text_x[b][idxs[b]]
        m_c = np.zeros((1, mv), np.float32)
        m_c[0, :nv] = 1.0
        in_maps.append(
            {
                "x": x_c,
                "t": t_c,
                "mask": m_c,
                "G1": G1,
                "g1b": g1b,
                "Wv": Wv2,
                "bv": bv2,
                "g2b": g2b,
                "Wcv": Wcv2,
                "bcv": bcv2,
            }
        )
    return (mv, mq), in_maps, idxs, fill, bffn[0]


def finish_output(dev_outs, idxs, fill, bffn):
    """Normalize + scatter compacted device outputs to [B, M, C].

    dev_outs[b] is [mv, C+1]: columns 0:C are the unnormalized attention-2
    numerator (with Wffn folded in); column C is the per-row exp-sum.
    """
    B = len(idxs)
    out = np.empty((B, M, C), np.float32)
    for b in range(B):
        nv = len(idxs[b])
        out[b, :, :] = fill[b]
        raw = dev_outs[b][:nv, :C].astype(np.float64)
        rs = dev_outs[b][:nv, C].astype(np.float64)
        out[b, idxs[b], :] = raw / rs[:, None] + bffn.astype(np.float64)
    return out


def kernel(
    layout_x, text_x, mask, Wqkv, bqkv, Wq, bq, Wkv, bkv, Wffn, bffn
):
    dims, in_maps, idxs, fill, bffn_v = prep_inputs(
        layout_x, text_x, mask, Wqkv, bqkv, Wq, bq, Wkv, bkv, Wffn, bffn
    )
    nc = _get_nc(dims)
    res = run_bass_kernel_spmd(nc, in_maps, core_ids=list(range(N_CORES)))
    return finish_output(
        [res.results[b]["out"] for b in range(N_CORES)], idxs, fill, bffn_v
    )
